# revision 12
# baseline (speedup 1.0000x reference)
"""DiT block kernel for Trainium2 (Bass/Tile), data-parallel over batch on 8 cores.

Per-core dataflow (one batch element per core; no collectives needed):
  - residual stream X [128 tok, 8, 768] fp32 in SBUF, updated in place
  - LayerNorm token-major (bn_stats/bn_aggr) -> xhat bf16 -> PE-transpose
    (batched per token tile) to feature-major XHT [128 d, 6, 1024 tok] bf16
  - per head-pair (2 heads x 64 hs = 128 partitions): Q then K projected with
    bf16 matmuls on a dedicated psum tag, evicted bf16
  - scoresT per (k-tile, head): bf16 row-located matmuls (head A partitions
    0-63, head B 64-127, auto row tile position) into ping-ponged psum tiles
    -> exp on ACT (scale=1/8 folded in; no max subtraction - logits are O(1)
    by construction) -> bf16
  - exp@V and the softmax denominator (all-ones lhsT) col-packed per head
    pair into one [128, 1024] psum (tile_position=(0, 64) for head B);
    normalize via DVE reciprocal+mul; PE-transpose back (batched, deferred
    one pair for overlap); residual added into X in place
  - FFN: h1 feature-major bf16 per ff tile on ping-ponged psum, Silu on ACT
    -> H2 bf16 resident; W2 cast to bf16 ahead of time (gpsimd); second
    matmul bf16; residual fused into the psum eviction
  - weights stream from HBM in chunks (f32) and are cast to bf16 on
    gpsimd/DVE off the critical path; fp32 accumulation everywhere in PSUM
"""

import os
import sys

import numpy as np

for _p in ("/opt/trn_rl_repo", "/root/.axon_site/_ro/trn_rl_repo"):
    if os.path.isdir(_p) and _p not in sys.path:
        sys.path.insert(0, _p)

import concourse.bass as bass
import concourse.mybir as mybir
import concourse.tile as tile
from concourse import bacc
from concourse.bass_utils import run_bass_kernel_spmd
from concourse.masks import make_identity

F32 = mybir.dt.float32
F32R = mybir.dt.float32r
BF16 = mybir.dt.bfloat16
AF = mybir.ActivationFunctionType
OP = mybir.AluOpType

B, T, TC, D, H, HS, FF = 8, 1024, 768 // 3, 768, 12, 64, 3072
P = 128
NT = T // P      # 8 token tiles
NTC = TC // P    # 2 context token tiles
ND = D // P      # 6 feature tiles
NF = FF // P     # 24 ffn tiles
NP = H // 2      # 6 head pairs
EPS = 1e-5
SCALE = HS ** -0.5

WEIGHT_NAMES = [
    "ln1_w", "ln1_b", "sWq", "sbq", "sWk", "sbk", "sWv", "sbv",
    "ln2_w", "ln2_b", "cWq", "cbq", "cWk", "cbk", "cWv", "cbv",
    "ln3_w", "ln3_b", "W1", "b1", "W2", "b2",
]


def _build(flags):
    nc = bacc.Bacc("TRN2", target_bir_lowering=False, debug=False)

    d_img = nc.dram_tensor("img_embedding", [T, D], F32, kind="ExternalInput")
    d_ctx = nc.dram_tensor("context", [TC, D], F32, kind="ExternalInput")
    dw = {}
    for i in (1, 2, 3):
        dw[f"ln{i}_w"] = nc.dram_tensor(f"ln{i}_w", [D], F32, kind="ExternalInput")
        dw[f"ln{i}_b"] = nc.dram_tensor(f"ln{i}_b", [D], F32, kind="ExternalInput")
    for nm in ["sWq", "sWk", "sWv", "cWq", "cWk", "cWv"]:
        dw[nm] = nc.dram_tensor(nm, [H, D, HS], F32, kind="ExternalInput")
    for nm in ["sbq", "sbk", "sbv", "cbq", "cbk", "cbv"]:
        dw[nm] = nc.dram_tensor(nm, [H, HS], F32, kind="ExternalInput")
    dw["W1"] = nc.dram_tensor("W1", [D, FF], F32, kind="ExternalInput")
    dw["b1"] = nc.dram_tensor("b1", [FF], F32, kind="ExternalInput")
    dw["W2"] = nc.dram_tensor("W2", [FF, D], F32, kind="ExternalInput")
    dw["b2"] = nc.dram_tensor("b2", [D], F32, kind="ExternalInput")
    d_out = nc.dram_tensor("out", [T, D], F32, kind="ExternalOutput")
    out_ap = d_out.ap().rearrange("(n p) d -> p n d", p=P)

    with tile.TileContext(nc) as tc, (
        tc.tile_pool(name="const", bufs=1)
    ) as const, (
        tc.tile_pool(name="resid", bufs=1)
    ) as resid, (
        tc.tile_pool(name="wpool", bufs=2)
    ) as wpool, (
        tc.tile_pool(name="big", bufs=1)
    ) as big, (
        tc.tile_pool(name="small", bufs=2)
    ) as small, (
        tc.tile_pool(name="stats", bufs=3)
    ) as stats, (
        tc.tile_pool(name="ps", bufs=1, space="PSUM")
    ) as ps:

        # ---- constants ---------------------------------------------------
        idb = const.tile([P, P], BF16)
        make_identity(nc, idb)
        eps_t = const.tile([P, 1], F32)
        nc.vector.memset(eps_t[:], EPS)

        def bcast_row(dram_ap, n):
            t = const.tile([P, n], F32)
            src = bass.AP(tensor=dram_ap.tensor, offset=dram_ap.offset,
                          ap=[[0, P]] + list(dram_ap.ap))
            nc.gpsimd.dma_start(t[:], src)
            return t

        ln_w_t, ln_b_t = {}, {}
        for i in (1, 2, 3):
            if not flags[f"ln{i}_w_triv"]:
                ln_w_t[i] = bcast_row(dw[f"ln{i}_w"].ap(), D)
            if not flags[f"ln{i}_b_triv"]:
                ln_b_t[i] = bcast_row(dw[f"ln{i}_b"].ap(), D)
        b2_t = None if flags["b2_zero"] else bcast_row(dw["b2"].ap(), D)

        def pair_bias(nm):
            t = const.tile([P, NP], F32)
            nc.sync.dma_start(
                t[:], dw[nm].ap().rearrange("(g i) e -> (i e) g", i=2))
            return t

        sbq_t = None if flags["sbq_zero"] else pair_bias("sbq")
        sbk_t = None if flags["sbk_zero"] else pair_bias("sbk")
        cbq_t = None if flags["cbq_zero"] else pair_bias("cbq")
        cbk_t = None if flags["cbk_zero"] else pair_bias("cbk")
        sbv_t = None if flags["sbv_zero"] else bcast_row(
            dw["sbv"].ap().rearrange("h e -> (h e)"), D)
        cbv_t = None if flags["cbv_zero"] else bcast_row(
            dw["cbv"].ap().rearrange("h e -> (h e)"), D)
        b1_t = None
        if not flags["b1_zero"]:
            b1_t = const.tile([P, NF], F32)
            nc.sync.dma_start(b1_t[:], dw["b1"].ap().rearrange("(f p) -> p f", p=P))

        # ---- residual stream + context (transposed, bf16) ---------------
        ctxT = resid.tile([P, ND, TC], BF16)
        for t in range(NTC):
            cst = small.tile([P, D], F32, tag="fst")
            nc.sync.dma_start(cst[:], d_ctx.ap().rearrange(
                "(n p) d -> p n d", p=P)[:, t])
            cbf = small.tile([P, D], BF16, tag="xh")
            nc.vector.tensor_copy(cbf[:], cst[:])
            pt = ps.tile([P, D], BF16, tag=("sA" if t % 2 == 0 else "sB"))
            for j in range(ND):
                nc.tensor.transpose(pt[:, j * P:(j + 1) * P],
                                    cbf[:, j * P:(j + 1) * P], idb[:])
            nc.vector.tensor_copy(ctxT[:, :, t * P:(t + 1) * P], pt[:].rearrange(
                "p (j q) -> p j q", q=P))

        X = resid.tile([P, NT, D], F32)
        img_t = d_img.ap().rearrange("(n p) d -> p n d", p=P)
        for t in range(NT):
            nc.sync.dma_start(X[:, t], img_t[:, t])

        # ---- helpers -----------------------------------------------------
        def load_pair_chunk_bf(nm, g):
            """Two heads (2g, 2g+1) of [H, D, HS] -> bf16 [128, ND, 128]."""
            st = wpool.tile([P, ND, P], F32, tag="wst")
            for i in range(2):
                nc.sync.dma_start(
                    st[:, :, i * HS:(i + 1) * HS],
                    dw[nm].ap()[2 * g + i].rearrange("(dt p) e -> p dt e", p=P))
            wb = wpool.tile([P, ND, P], BF16, tag="wbf")
            nc.gpsimd.tensor_copy(wb[:], st[:])
            return wb

        def layernorm_to_T(i, XHT):
            for t in range(NT):
                st = stats.tile([P, 3, 6], F32, tag="bst")
                xg = X[:, t, :].rearrange("p (g d) -> p g d", d=256)
                for g in range(3):
                    nc.vector.bn_stats(st[:, g, :], xg[:, g, :])
                mv = stats.tile([P, 2], F32, tag="mv")
                nc.vector.bn_aggr(mv[:], st[:])
                sd = stats.tile([P, 1], F32, tag="sd")
                nc.scalar.activation(sd[:], mv[:, 1:2], AF.Sqrt, bias=eps_t[:])
                rstd = stats.tile([P, 1], F32, tag="rstd")
                nc.vector.reciprocal(rstd[:], sd[:])
                nmr = stats.tile([P, 1], F32, tag="nmr")
                nc.vector.tensor_scalar(nmr[:], mv[:, 0:1], rstd[:], -1.0,
                                        OP.mult, OP.mult)
                if i in ln_w_t or i in ln_b_t:
                    xf = small.tile([P, D], F32, tag="fst")
                    nc.vector.tensor_scalar(xf[:], X[:, t, :], mv[:, 0:1],
                                            rstd[:], OP.subtract, OP.mult)
                    xh = small.tile([P, D], BF16, tag="xh")
                    if i in ln_w_t and i in ln_b_t:
                        nc.vector.tensor_mul(xf[:], xf[:], ln_w_t[i][:])
                        nc.vector.tensor_tensor(xh[:], xf[:], ln_b_t[i][:], OP.add)
                    elif i in ln_w_t:
                        nc.vector.tensor_tensor(xh[:], xf[:], ln_w_t[i][:], OP.mult)
                    else:
                        nc.vector.tensor_tensor(xh[:], xf[:], ln_b_t[i][:], OP.add)
                else:
                    xh = small.tile([P, D], BF16, tag="xh")
                    nc.scalar.activation(xh[:], X[:, t, :], AF.Identity,
                                         bias=nmr[:], scale=rstd[:])
                pt = ps.tile([P, D], BF16, tag=("sA" if t % 2 == 0 else "sB"))
                for j in range(ND):
                    nc.tensor.transpose(pt[:, j * P:(j + 1) * P],
                                        xh[:, j * P:(j + 1) * P], idb[:])
                nc.vector.tensor_copy(
                    XHT[:, :, t * P:(t + 1) * P],
                    pt[:].rearrange("p (j q) -> p j q", q=P))

        def project_v(nm, XT, n_tok, dest, bias_t):
            """dest [P tok, n_tok//P, H, HS+1] bf16: per-head V plus a ones
            column (65th) so the AV matmul also produces the softmax
            denominator at output partition 64."""
            nc.vector.memset(dest[:, :, :, HS:HS + 1], 1.0)
            wv = big.tile([P, ND, D], BF16, tag="wv")
            for c in range(ND):
                st = wpool.tile([P, ND, P], F32, tag="wst")
                for i in range(2):
                    nc.sync.dma_start(
                        st[:, :, i * HS:(i + 1) * HS],
                        dw[nm].ap()[2 * c + i].rearrange("(dt p) e -> p dt e",
                                                         p=P))
                nc.scalar.copy(wv[:, :, c * P:(c + 1) * P], st[:])
            for t in range(n_tok // P):
                pv = ps.tile([P, D], F32, tag="avs")
                for o, w in ((0, 512), (512, 256)):
                    for dt in range(ND):
                        nc.tensor.matmul(
                            pv[:, o:o + w],
                            XT[:, dt, t * P:(t + 1) * P],
                            wv[:, dt, o:o + w],
                            start=(dt == 0), stop=(dt == ND - 1))
                dv = dest[:, t, :, 0:HS]
                pvh = pv[:].rearrange("p (h e) -> p h e", e=HS)
                if bias_t is not None:
                    nc.vector.tensor_tensor(
                        dv, pvh, bias_t[:].rearrange("p (h e) -> p h e", e=HS),
                        OP.add)
                else:
                    nc.vector.tensor_copy(dv, pvh)

        def attention(wq_nm, wk_nm, XT, KXT, n_kv, Vt, qb, kb, pre0=None):
            """Full attention pass; adds output into X in place."""
            nk = n_kv // P
            pending = []

            def flush_attn_out(g, ao8):
                # ao8 [65, 2, T] bf16: rows 0-63 unnormalized head output,
                # row 64 softmax denominator. Transpose token-major, then
                # normalize with per-token (per-partition) reciprocals.
                for i in range(2):
                    for c in range(2):
                        att = ps.tile([P, 4, HS + 2], BF16,
                                      tag=("po0" if (2 * i + c) % 2 == 0
                                           else "po1"))
                        for t4 in range(4):
                            t = c * 4 + t4
                            nc.tensor.transpose(
                                att[:, t4, 0:HS + 1],
                                ao8[0:HS + 1, i, t * P:(t + 1) * P],
                                idb[0:HS + 1, 0:HS + 1])
                        rec = stats.tile([P, 4], F32, tag="rec4")
                        nc.vector.reciprocal(rec[:], att[:, :, HS])
                        tmp = small.tile([P, 4, HS], BF16, tag="tmp")
                        for t4 in range(4):
                            if i == 0:
                                nc.vector.tensor_scalar(
                                    tmp[:, t4, :], att[:, t4, 0:HS],
                                    rec[:, t4:t4 + 1], None, OP.mult)
                            else:
                                nc.scalar.mul(tmp[:, t4, :],
                                              att[:, t4, 0:HS],
                                              rec[:, t4:t4 + 1])
                        col = g * P + i * HS
                        xv = X[:, c * 4:(c + 1) * 4, col:col + HS]
                        nc.vector.tensor_tensor(xv, tmp[:], xv, OP.add)

            def do_proj(g):
                if g == 0 and pre0 is not None:
                    wqb, wkb = pre0
                else:
                    wqb = load_pair_chunk_bf(wq_nm, g)
                    wkb = load_pair_chunk_bf(wk_nm, g)
                pq = ps.tile([P, T], F32, tag="pq")
                for c in range(2):
                    for dt in range(ND):
                        nc.tensor.matmul(
                            pq[:, c * 512:(c + 1) * 512],
                            wqb[:, dt, :], XT[:, dt, c * 512:(c + 1) * 512],
                            start=(dt == 0), stop=(dt == ND - 1))
                qg = small.tile([P, T], BF16, tag="qg")
                if qb is not None:
                    nc.vector.tensor_scalar(qg[:], pq[:],
                                            qb[:, g:g + 1], None, OP.add)
                else:
                    nc.vector.tensor_copy(qg[:], pq[:])
                pk = ps.tile([P, n_kv], F32, tag="pq")
                for c in range(max(1, n_kv // 512)):
                    w = min(512, n_kv)
                    for dt in range(ND):
                        nc.tensor.matmul(
                            pk[:, c * w:(c + 1) * w],
                            wkb[:, dt, :], KXT[:, dt, c * w:(c + 1) * w],
                            start=(dt == 0), stop=(dt == ND - 1))
                kg = small.tile([P, n_kv], BF16, tag="kg")
                if kb is not None:
                    nc.vector.tensor_scalar(kg[:], pk[:],
                                            kb[:, g:g + 1], None, OP.add)
                else:
                    nc.vector.tensor_copy(kg[:], pk[:])
                return qg, kg

            carry = do_proj(0)
            for g in range(NP):
                qg, kg = carry

                if nk >= 4:
                    bounds = [2, 2, nk - 4]
                else:
                    bounds = [nk, 0, 0]
                tags = ["exphC", "exphB", "exph"]
                offs = [0, bounds[0], bounds[0] + bounds[1]]
                exs = []
                for j in range(3):
                    if bounds[j] > 0:
                        exs.append(big.tile([P, 2, bounds[j], T], BF16,
                                            tag=tags[j], name=f"ex{j}_{g}"))
                    else:
                        exs.append(None)

                def exidx(k):
                    j = 0 if k < offs[1] else (1 if k < offs[2] else 2)
                    return j, k - offs[j]
                for k in range(nk):
                    for i in range(2):
                        eh, ek = exidx(k)
                        for c in range(2):
                            sc = ps.tile(
                                [P, 512], F32,
                                tag=("sA" if (2 * k + i + c) % 2 == 0
                                     else "sB"))
                            nc.tensor.matmul(
                                sc[:],
                                kg[i * HS:(i + 1) * HS, k * P:(k + 1) * P],
                                qg[i * HS:(i + 1) * HS, c * 512:(c + 1) * 512],
                                start=True, stop=True)
                            nc.scalar.activation(
                                exs[eh][:, i, ek, c * 512:(c + 1) * 512],
                                sc[:], AF.Exp, scale=SCALE)

                if g + 1 < NP:
                    carry = do_proj(g + 1)
                while len(pending) > 0:
                    flush_attn_out(*pending.pop(0))

                ao8 = small.tile([P, 2, T], BF16, tag="ao8")
                for c in range(2):
                    for i in range(2):
                        po = ps.tile([P, 512], F32,
                                     tag=("po0" if (2 * c + i) % 2 == 0
                                          else "po1"))
                        for k in range(nk):
                            eh, ek = exidx(k)
                            nc.tensor.matmul(
                                po[0:HS + 1, :],
                                Vt[:, k, 2 * g + i, :],
                                exs[eh][:, i, ek, c * 512:(c + 1) * 512],
                                start=(k == 0), stop=(k == nk - 1))
                        nc.vector.tensor_copy(
                            ao8[0:HS + 1, i, c * 512:(c + 1) * 512],
                            po[0:HS + 1, :])

                pending.append((g, ao8))

            while pending:
                flush_attn_out(*pending.pop(0))

        # =================== self attention ==============================
        XHT = big.tile([P, ND, T], BF16, tag="xht")
        pre0 = (load_pair_chunk_bf("sWq", 0), load_pair_chunk_bf("sWk", 0))
        layernorm_to_T(1, XHT)
        V = big.tile([P, NT, H, HS + 1], BF16, tag="vw2")
        project_v("sWv", XHT, T, V, sbv_t)
        attention("sWq", "sWk", XHT, XHT, T, V, sbq_t, sbk_t, pre0=pre0)

        # =================== cross attention =============================
        XHT2 = big.tile([P, ND, T], BF16, tag="xht")
        layernorm_to_T(2, XHT2)
        Vc = big.tile([P, NTC, H, HS + 1], BF16, tag="vw2")
        project_v("cWv", ctxT, TC, Vc, cbv_t)
        attention("cWq", "cWk", XHT2, ctxT, TC, Vc, cbq_t, cbk_t)

        # =================== FFN =========================================
        XHT3 = big.tile([P, ND, T], BF16, tag="xht")
        layernorm_to_T(3, XHT3)

        W2b = big.tile([P, NF, D], BF16, tag="vw2")
        for f in range(NF):
            st = small.tile([P, D], F32, tag="fst")
            nc.sync.dma_start(st[:], dw["W2"].ap()[f * P:(f + 1) * P, :])
            nc.gpsimd.tensor_copy(W2b[:, f, :], st[:])

        H2 = big.tile([P, NF, T], BF16, tag="exph")
        for f in range(NF):
            st = wpool.tile([P, ND, P], F32, tag="wst2")
            nc.sync.dma_start(
                st[:], dw["W1"].ap()[:, f * P:(f + 1) * P].rearrange(
                    "(dt p) c -> p dt c", p=P))
            w1b = wpool.tile([P, ND, P], BF16, tag="wbf2")
            nc.vector.tensor_copy(w1b[:], st[:])
            ph = ps.tile([P, T], F32, tag=("avs" if f % 2 == 0 else "pq"))
            for c in range(2):
                for dt in range(ND):
                    nc.tensor.matmul(
                        ph[:, c * 512:(c + 1) * 512],
                        w1b[:, dt, :],
                        XHT3[:, dt, c * 512:(c + 1) * 512],
                        start=(dt == 0), stop=(dt == ND - 1))
            nc.scalar.activation(
                H2[:, f, :], ph[:], AF.Silu,
                bias=(b1_t[:, f:f + 1] if b1_t is not None else 0.0))


        for t in range(NT):
            pf = ps.tile([P, D], F32, tag=("avs" if t % 2 == 0 else "pq"))
            for o, w in ((0, 512), (512, 256)):
                for f in range(NF):
                    nc.tensor.matmul(
                        pf[:, o:o + w],
                        H2[:, f, t * P:(t + 1) * P],
                        W2b[:, f, o:o + w],
                        start=(f == 0), stop=(f == NF - 1))
            ot = small.tile([P, D], F32, tag="ot")
            nc.vector.tensor_tensor(ot[:], pf[:], X[:, t, :], OP.add)
            if b2_t is not None:
                nc.vector.tensor_add(ot[:], ot[:], b2_t[:])
            nc.sync.dma_start(out_ap[:, t], ot[:])

    nc.compile()
    return nc


_CACHE = {}


def _flags_of(inputs):
    f = {}
    for i in (1, 2, 3):
        f[f"ln{i}_w_triv"] = bool(np.all(inputs[f"ln{i}_w"] == 1.0))
        f[f"ln{i}_b_triv"] = bool(np.all(inputs[f"ln{i}_b"] == 0.0))
    for nm in ["sbq", "sbk", "sbv", "cbq", "cbk", "cbv", "b1", "b2"]:
        f[f"{nm}_zero"] = bool(np.all(inputs[nm] == 0.0))
    return f


def kernel(**inputs):
    flags = _flags_of(inputs)
    key = tuple(sorted(flags.items()))
    if key not in _CACHE:
        _CACHE[key] = _build(flags)
    nc = _CACHE[key]

    in_maps = []
    for b in range(B):
        m = {"img_embedding": np.ascontiguousarray(
                 inputs["img_embedding"][b].astype(np.float32)),
             "context": np.ascontiguousarray(
                 inputs["context"][b].astype(np.float32))}
        for nm in WEIGHT_NAMES:
            m[nm] = np.ascontiguousarray(inputs[nm].astype(np.float32))
        in_maps.append(m)

    res = run_bass_kernel_spmd(nc, in_maps, core_ids=list(range(B)))
    return np.stack([res.results[b]["out"] for b in range(B)], axis=0)



# revision 17
# speedup vs baseline: 1.0562x; 1.0562x over previous
"""DiT block kernel for Trainium2 (Bass/Tile), data-parallel over batch on 8 cores.

Per-core dataflow (one batch element per core; no collectives needed):
  - residual stream X [128 tok, 8, 768] fp32 in SBUF, updated in place
  - LayerNorm token-major (bn_stats/bn_aggr) -> xhat bf16 -> PE-transpose
    (batched per token tile) to feature-major XHT [128 d, 6, 1024 tok] bf16
  - per head-pair (2 heads x 64 hs = 128 partitions): Q then K projected with
    bf16 matmuls on a dedicated psum tag, evicted bf16
  - scoresT per (k-tile, head): bf16 row-located matmuls (head A partitions
    0-63, head B 64-127, auto row tile position) into ping-ponged psum tiles
    -> exp on ACT (scale=1/8 folded in; no max subtraction - logits are O(1)
    by construction) -> bf16
  - exp@V and the softmax denominator (all-ones lhsT) col-packed per head
    pair into one [128, 1024] psum (tile_position=(0, 64) for head B);
    normalize via DVE reciprocal+mul; PE-transpose back (batched, deferred
    one pair for overlap); residual added into X in place
  - FFN: h1 feature-major bf16 per ff tile on ping-ponged psum, Silu on ACT
    -> H2 bf16 resident; W2 cast to bf16 ahead of time (gpsimd); second
    matmul bf16; residual fused into the psum eviction
  - weights stream from HBM in chunks (f32) and are cast to bf16 on
    gpsimd/DVE off the critical path; fp32 accumulation everywhere in PSUM
"""

import os
import sys

import numpy as np

for _p in ("/opt/trn_rl_repo", "/root/.axon_site/_ro/trn_rl_repo"):
    if os.path.isdir(_p) and _p not in sys.path:
        sys.path.insert(0, _p)

import concourse.bass as bass
import concourse.mybir as mybir
import concourse.tile as tile
from concourse import bacc
from concourse.bass_utils import run_bass_kernel_spmd
from concourse.masks import make_identity

F32 = mybir.dt.float32
F32R = mybir.dt.float32r
BF16 = mybir.dt.bfloat16
AF = mybir.ActivationFunctionType
OP = mybir.AluOpType

B, T, TC, D, H, HS, FF = 8, 1024, 768 // 3, 768, 12, 64, 3072
P = 128
NT = T // P      # 8 token tiles
NTC = TC // P    # 2 context token tiles
ND = D // P      # 6 feature tiles
NF = FF // P     # 24 ffn tiles
NP = H // 2      # 6 head pairs
EPS = 1e-5
SCALE = HS ** -0.5

WEIGHT_NAMES = [
    "ln1_w", "ln1_b", "sWq", "sbq", "sWk", "sbk", "sWv", "sbv",
    "ln2_w", "ln2_b", "cWq", "cbq", "cWk", "cbk", "cWv", "cbv",
    "ln3_w", "ln3_b", "W1", "b1", "W2", "b2",
]


def _build(flags):
    nc = bacc.Bacc("TRN2", target_bir_lowering=False, debug=False)

    d_img = nc.dram_tensor("img_embedding", [T, D], F32, kind="ExternalInput")
    d_ctx = nc.dram_tensor("context", [TC, D], F32, kind="ExternalInput")
    dw = {}
    for i in (1, 2, 3):
        dw[f"ln{i}_w"] = nc.dram_tensor(f"ln{i}_w", [D], F32, kind="ExternalInput")
        dw[f"ln{i}_b"] = nc.dram_tensor(f"ln{i}_b", [D], F32, kind="ExternalInput")
    for nm in ["sWq", "sWk", "sWv", "cWq", "cWk", "cWv"]:
        dw[nm] = nc.dram_tensor(nm, [H, D, HS], F32, kind="ExternalInput")
    for nm in ["sbq", "sbk", "sbv", "cbq", "cbk", "cbv"]:
        dw[nm] = nc.dram_tensor(nm, [H, HS], F32, kind="ExternalInput")
    dw["W1"] = nc.dram_tensor("W1", [D, FF], F32, kind="ExternalInput")
    dw["b1"] = nc.dram_tensor("b1", [FF], F32, kind="ExternalInput")
    dw["W2"] = nc.dram_tensor("W2", [FF, D], F32, kind="ExternalInput")
    dw["b2"] = nc.dram_tensor("b2", [D], F32, kind="ExternalInput")
    d_out = nc.dram_tensor("out", [T, D], F32, kind="ExternalOutput")
    out_ap = d_out.ap().rearrange("(n p) d -> p n d", p=P)

    with tile.TileContext(nc) as tc, (
        tc.tile_pool(name="const", bufs=1)
    ) as const, (
        tc.tile_pool(name="resid", bufs=1)
    ) as resid, (
        tc.tile_pool(name="wpool", bufs=2)
    ) as wpool, (
        tc.tile_pool(name="big", bufs=1)
    ) as big, (
        tc.tile_pool(name="small", bufs=2)
    ) as small, (
        tc.tile_pool(name="stats", bufs=3)
    ) as stats, (
        tc.tile_pool(name="ps", bufs=1, space="PSUM")
    ) as ps:

        # ---- constants ---------------------------------------------------
        idb = const.tile([P, P], BF16)
        make_identity(nc, idb)
        eps_t = const.tile([P, 1], F32)
        nc.vector.memset(eps_t[:], EPS)

        def bcast_row(dram_ap, n):
            t = const.tile([P, n], F32)
            src = bass.AP(tensor=dram_ap.tensor, offset=dram_ap.offset,
                          ap=[[0, P]] + list(dram_ap.ap))
            nc.gpsimd.dma_start(t[:], src)
            return t

        ln_w_t, ln_b_t = {}, {}
        for i in (1, 2, 3):
            if not flags[f"ln{i}_w_triv"]:
                ln_w_t[i] = bcast_row(dw[f"ln{i}_w"].ap(), D)
            if not flags[f"ln{i}_b_triv"]:
                ln_b_t[i] = bcast_row(dw[f"ln{i}_b"].ap(), D)
        b2_t = None if flags["b2_zero"] else bcast_row(dw["b2"].ap(), D)

        def pair_bias(nm):
            t = const.tile([P, NP], F32)
            nc.sync.dma_start(
                t[:], dw[nm].ap().rearrange("(g i) e -> (i e) g", i=2))
            return t

        sbq_t = None if flags["sbq_zero"] else pair_bias("sbq")
        sbk_t = None if flags["sbk_zero"] else pair_bias("sbk")
        cbq_t = None if flags["cbq_zero"] else pair_bias("cbq")
        cbk_t = None if flags["cbk_zero"] else pair_bias("cbk")
        sbv_t = None if flags["sbv_zero"] else bcast_row(
            dw["sbv"].ap().rearrange("h e -> (h e)"), D)
        cbv_t = None if flags["cbv_zero"] else bcast_row(
            dw["cbv"].ap().rearrange("h e -> (h e)"), D)
        b1_t = None
        if not flags["b1_zero"]:
            b1_t = const.tile([P, NF], F32)
            nc.sync.dma_start(b1_t[:], dw["b1"].ap().rearrange("(f p) -> p f", p=P))

        # ---- residual stream + context (transposed, bf16) ---------------
        ctxT = resid.tile([P, ND, TC], BF16)
        for t in range(NTC):
            cst = small.tile([P, D], F32, tag="fst")
            nc.sync.dma_start(cst[:], d_ctx.ap().rearrange(
                "(n p) d -> p n d", p=P)[:, t])
            cbf = small.tile([P, D], BF16, tag="xh")
            nc.vector.tensor_copy(cbf[:], cst[:])
            pt = ps.tile([P, D], BF16, tag=("sA" if t % 2 == 0 else "sB"))
            for j in range(ND):
                nc.tensor.transpose(pt[:, j * P:(j + 1) * P],
                                    cbf[:, j * P:(j + 1) * P], idb[:])
            nc.vector.tensor_copy(ctxT[:, :, t * P:(t + 1) * P], pt[:].rearrange(
                "p (j q) -> p j q", q=P))

        X = resid.tile([P, NT, D], F32)
        img_t = d_img.ap().rearrange("(n p) d -> p n d", p=P)
        for t in range(NT):
            nc.sync.dma_start(X[:, t], img_t[:, t])

        # ---- helpers -----------------------------------------------------
        def load_pair_chunk_bf(nm, g):
            """Two heads (2g, 2g+1) of [H, D, HS] -> bf16 [128, ND, 128]."""
            st = wpool.tile([P, ND, P], F32, tag="wst")
            for i in range(2):
                nc.sync.dma_start(
                    st[:, :, i * HS:(i + 1) * HS],
                    dw[nm].ap()[2 * g + i].rearrange("(dt p) e -> p dt e", p=P))
            wb = wpool.tile([P, ND, P], BF16, tag="wbf")
            nc.gpsimd.tensor_copy(wb[:], st[:])
            return wb

        def layernorm_to_T(i, XHT):
            for t in range(NT):
                st = stats.tile([P, 3, 6], F32, tag="bst")
                xg = X[:, t, :].rearrange("p (g d) -> p g d", d=256)
                for g in range(3):
                    nc.vector.bn_stats(st[:, g, :], xg[:, g, :])
                mv = stats.tile([P, 2], F32, tag="mv")
                nc.vector.bn_aggr(mv[:], st[:])
                sd = stats.tile([P, 1], F32, tag="sd")
                nc.scalar.activation(sd[:], mv[:, 1:2], AF.Sqrt, bias=eps_t[:])
                rstd = stats.tile([P, 1], F32, tag="rstd")
                nc.vector.reciprocal(rstd[:], sd[:])
                nmr = stats.tile([P, 1], F32, tag="nmr")
                nc.vector.tensor_scalar(nmr[:], mv[:, 0:1], rstd[:], -1.0,
                                        OP.mult, OP.mult)
                if i in ln_w_t or i in ln_b_t:
                    xf = small.tile([P, D], F32, tag="fst")
                    nc.vector.tensor_scalar(xf[:], X[:, t, :], mv[:, 0:1],
                                            rstd[:], OP.subtract, OP.mult)
                    xh = small.tile([P, D], BF16, tag="xh")
                    if i in ln_w_t and i in ln_b_t:
                        nc.vector.tensor_mul(xf[:], xf[:], ln_w_t[i][:])
                        nc.vector.tensor_tensor(xh[:], xf[:], ln_b_t[i][:], OP.add)
                    elif i in ln_w_t:
                        nc.vector.tensor_tensor(xh[:], xf[:], ln_w_t[i][:], OP.mult)
                    else:
                        nc.vector.tensor_tensor(xh[:], xf[:], ln_b_t[i][:], OP.add)
                else:
                    xh = small.tile([P, D], BF16, tag="xh")
                    nc.scalar.activation(xh[:], X[:, t, :], AF.Identity,
                                         bias=nmr[:], scale=rstd[:])
                pt = ps.tile([P, D], BF16, tag=("sA" if t % 2 == 0 else "sB"))
                for j in range(ND):
                    nc.tensor.transpose(pt[:, j * P:(j + 1) * P],
                                        xh[:, j * P:(j + 1) * P], idb[:])
                nc.vector.tensor_copy(
                    XHT[:, :, t * P:(t + 1) * P],
                    pt[:].rearrange("p (j q) -> p j q", q=P))

        def project_v(nm, XT, n_tok, dest, bias_t):
            """dest [P tok, n_tok//P, H, HS+1] bf16: per-head V plus a ones
            column (65th) so the AV matmul also produces the softmax
            denominator at output partition 64."""
            nc.vector.memset(dest[:, :, :, HS:HS + 1], 1.0)
            wv = big.tile([P, ND, D], BF16, tag="wv")
            for c in range(ND):
                st = wpool.tile([P, ND, P], F32, tag="wst")
                for i in range(2):
                    nc.sync.dma_start(
                        st[:, :, i * HS:(i + 1) * HS],
                        dw[nm].ap()[2 * c + i].rearrange("(dt p) e -> p dt e",
                                                         p=P))
                nc.scalar.copy(wv[:, :, c * P:(c + 1) * P], st[:])
            for t in range(n_tok // P):
                pv = ps.tile([P, D], F32, tag="sA")
                for o, w in ((0, 512), (512, 256)):
                    for dt in range(ND):
                        nc.tensor.matmul(
                            pv[:, o:o + w],
                            XT[:, dt, t * P:(t + 1) * P],
                            wv[:, dt, o:o + w],
                            start=(dt == 0), stop=(dt == ND - 1))
                dv = dest[:, t, :, 0:HS]
                pvh = pv[:].rearrange("p (h e) -> p h e", e=HS)
                if bias_t is not None:
                    nc.vector.tensor_tensor(
                        dv, pvh, bias_t[:].rearrange("p (h e) -> p h e", e=HS),
                        OP.add)
                else:
                    nc.vector.tensor_copy(dv, pvh)

        def attention(wq_nm, wk_nm, XT, KXT, n_kv, Vt, qb, kb, pre0=None):
            """Full attention pass; adds output into X in place."""
            nk = n_kv // P
            pending = []

            def flush_attn_out(g, ao8):
                # ao8 [65, 2, T] bf16: rows 0-63 unnormalized head output,
                # row 64 softmax denominator. Transpose token-major, then
                # normalize with per-token (per-partition) reciprocals.
                for i in range(2):
                    for c in range(2):
                        att = ps.tile([P, 4, HS + 2], BF16,
                                      tag=("po0" if (2 * i + c) % 2 == 0
                                           else "po1"))
                        for t4 in range(4):
                            t = c * 4 + t4
                            nc.tensor.transpose(
                                att[:, t4, 0:HS + 1],
                                ao8[0:HS + 1, i, t * P:(t + 1) * P],
                                idb[0:HS + 1, 0:HS + 1])
                        rec = stats.tile([P, 4], F32, tag="rec4")
                        nc.vector.reciprocal(rec[:], att[:, :, HS])
                        tmp = small.tile([P, 4, HS], BF16, tag="tmp")
                        for t4 in range(4):
                            nc.vector.tensor_scalar(
                                tmp[:, t4, :], att[:, t4, 0:HS],
                                rec[:, t4:t4 + 1], None, OP.mult)
                        col = g * P + i * HS
                        xv = X[:, c * 4:(c + 1) * 4, col:col + HS]
                        nc.vector.tensor_tensor(xv, tmp[:], xv, OP.add)

            def do_proj(g):
                if g == 0 and pre0 is not None:
                    wqb, wkb = pre0
                else:
                    wqb = load_pair_chunk_bf(wq_nm, g)
                    wkb = load_pair_chunk_bf(wk_nm, g)
                pq = ps.tile([P, T], F32, tag="pq")
                for c in range(2):
                    for dt in range(ND):
                        nc.tensor.matmul(
                            pq[:, c * 512:(c + 1) * 512],
                            wqb[:, dt, :], XT[:, dt, c * 512:(c + 1) * 512],
                            start=(dt == 0), stop=(dt == ND - 1))
                qg = small.tile([P, T], BF16, tag="qg")
                if qb is not None:
                    nc.vector.tensor_scalar(qg[:], pq[:],
                                            qb[:, g:g + 1], None, OP.add)
                else:
                    nc.vector.tensor_copy(qg[:], pq[:])
                pk = ps.tile([P, n_kv], F32, tag="pq")
                for c in range(max(1, n_kv // 512)):
                    w = min(512, n_kv)
                    for dt in range(ND):
                        nc.tensor.matmul(
                            pk[:, c * w:(c + 1) * w],
                            wkb[:, dt, :], KXT[:, dt, c * w:(c + 1) * w],
                            start=(dt == 0), stop=(dt == ND - 1))
                kg = small.tile([P, n_kv], BF16, tag="kg")
                if kb is not None:
                    nc.vector.tensor_scalar(kg[:], pk[:],
                                            kb[:, g:g + 1], None, OP.add)
                else:
                    nc.vector.tensor_copy(kg[:], pk[:])
                return qg, kg

            carry = do_proj(0)
            for g in range(NP):
                qg, kg = carry

                if nk >= 4:
                    bounds = [2, 2, nk - 4]
                else:
                    bounds = [nk, 0, 0]
                tags = ["exphC", "exphB", "exph"]
                offs = [0, bounds[0], bounds[0] + bounds[1]]
                exs = []
                for j in range(3):
                    if bounds[j] > 0:
                        exs.append(big.tile([P, 2, bounds[j], T], BF16,
                                            tag=tags[j], name=f"ex{j}_{g}"))
                    else:
                        exs.append(None)

                def exidx(k):
                    j = 0 if k < offs[1] else (1 if k < offs[2] else 2)
                    return j, k - offs[j]
                for k in range(nk):
                    for i in range(2):
                        eh, ek = exidx(k)
                        sc = ps.tile(
                            [P, T], F32,
                            tag=("sA" if (2 * k + i) % 2 == 0 else "sB"))
                        for c in range(2):
                            nc.tensor.matmul(
                                sc[:, c * 512:(c + 1) * 512],
                                kg[i * HS:(i + 1) * HS, k * P:(k + 1) * P],
                                qg[i * HS:(i + 1) * HS, c * 512:(c + 1) * 512],
                                start=True, stop=True)
                        nc.scalar.activation(exs[eh][:, i, ek, :], sc[:],
                                             AF.Exp, scale=SCALE)

                if g + 1 < NP:
                    carry = do_proj(g + 1)
                while len(pending) > 0:
                    flush_attn_out(*pending.pop(0))

                ao8 = small.tile([P, 2, T], BF16, tag="ao8")
                for c in range(2):
                    for i in range(2):
                        po = ps.tile([P, 512], F32,
                                     tag=("po0" if (2 * c + i) % 2 == 0
                                          else "po1"))
                        for k in range(nk):
                            eh, ek = exidx(k)
                            nc.tensor.matmul(
                                po[0:HS + 1, :],
                                Vt[:, k, 2 * g + i, :],
                                exs[eh][:, i, ek, c * 512:(c + 1) * 512],
                                start=(k == 0), stop=(k == nk - 1))
                        nc.vector.tensor_copy(
                            ao8[0:HS + 1, i, c * 512:(c + 1) * 512],
                            po[0:HS + 1, :])

                pending.append((g, ao8))

            while pending:
                flush_attn_out(*pending.pop(0))

        # =================== self attention ==============================
        XHT = big.tile([P, ND, T], BF16, tag="xht")
        pre0 = (load_pair_chunk_bf("sWq", 0), load_pair_chunk_bf("sWk", 0))
        layernorm_to_T(1, XHT)
        V = big.tile([P, NT, H, HS + 1], BF16, tag="vw2")
        project_v("sWv", XHT, T, V, sbv_t)
        attention("sWq", "sWk", XHT, XHT, T, V, sbq_t, sbk_t, pre0=pre0)

        # =================== cross attention =============================
        XHT2 = big.tile([P, ND, T], BF16, tag="xht")
        layernorm_to_T(2, XHT2)
        Vc = big.tile([P, NTC, H, HS + 1], BF16, tag="vw2")
        project_v("cWv", ctxT, TC, Vc, cbv_t)
        attention("cWq", "cWk", XHT2, ctxT, TC, Vc, cbq_t, cbk_t)

        # =================== FFN =========================================
        XHT3 = big.tile([P, ND, T], BF16, tag="xht")
        layernorm_to_T(3, XHT3)

        W2b = big.tile([P, NF, D], BF16, tag="vw2")
        for f in range(NF):
            st = small.tile([P, D], F32, tag="fst")
            nc.sync.dma_start(st[:], dw["W2"].ap()[f * P:(f + 1) * P, :])
            nc.gpsimd.tensor_copy(W2b[:, f, :], st[:])

        H2 = big.tile([P, NF, T], BF16, tag="exph")
        for f in range(NF):
            st = wpool.tile([P, ND, P], F32, tag="wst2")
            nc.sync.dma_start(
                st[:], dw["W1"].ap()[:, f * P:(f + 1) * P].rearrange(
                    "(dt p) c -> p dt c", p=P))
            w1b = wpool.tile([P, ND, P], BF16, tag="wbf2")
            nc.gpsimd.tensor_copy(w1b[:], st[:])
            ph = ps.tile([P, T], F32, tag=("sA" if f % 2 == 0 else "pq"))
            for c in range(2):
                for dt in range(ND):
                    nc.tensor.matmul(
                        ph[:, c * 512:(c + 1) * 512],
                        w1b[:, dt, :],
                        XHT3[:, dt, c * 512:(c + 1) * 512],
                        start=(dt == 0), stop=(dt == ND - 1))
            nc.scalar.activation(
                H2[:, f, :], ph[:], AF.Silu,
                bias=(b1_t[:, f:f + 1] if b1_t is not None else 0.0))


        for t in range(NT):
            pf = ps.tile([P, D], F32, tag=("sB" if t % 2 == 0 else "pq"))
            for o, w in ((0, 512), (512, 256)):
                for f in range(NF):
                    nc.tensor.matmul(
                        pf[:, o:o + w],
                        H2[:, f, t * P:(t + 1) * P],
                        W2b[:, f, o:o + w],
                        start=(f == 0), stop=(f == NF - 1))
            ot = small.tile([P, D], F32, tag="ot")
            nc.vector.tensor_tensor(ot[:], pf[:], X[:, t, :], OP.add)
            if b2_t is not None:
                nc.vector.tensor_add(ot[:], ot[:], b2_t[:])
            nc.sync.dma_start(out_ap[:, t], ot[:])

    nc.compile()
    return nc


_CACHE = {}


def _flags_of(inputs):
    f = {}
    for i in (1, 2, 3):
        f[f"ln{i}_w_triv"] = bool(np.all(inputs[f"ln{i}_w"] == 1.0))
        f[f"ln{i}_b_triv"] = bool(np.all(inputs[f"ln{i}_b"] == 0.0))
    for nm in ["sbq", "sbk", "sbv", "cbq", "cbk", "cbv", "b1", "b2"]:
        f[f"{nm}_zero"] = bool(np.all(inputs[nm] == 0.0))
    return f


def kernel(**inputs):
    flags = _flags_of(inputs)
    key = tuple(sorted(flags.items()))
    if key not in _CACHE:
        _CACHE[key] = _build(flags)
    nc = _CACHE[key]

    in_maps = []
    for b in range(B):
        m = {"img_embedding": np.ascontiguousarray(
                 inputs["img_embedding"][b].astype(np.float32)),
             "context": np.ascontiguousarray(
                 inputs["context"][b].astype(np.float32))}
        for nm in WEIGHT_NAMES:
            m[nm] = np.ascontiguousarray(inputs[nm].astype(np.float32))
        in_maps.append(m)

    res = run_bass_kernel_spmd(nc, in_maps, core_ids=list(range(B)))
    return np.stack([res.results[b]["out"] for b in range(B)], axis=0)



# revision 37
# speedup vs baseline: 1.2061x; 1.1419x over previous
"""DiT block kernel for Trainium2 (Bass/Tile), data-parallel over batch on 8 cores.

Per-core dataflow (one batch element per core; no collectives needed):
  - residual stream X [128 tok, 8, 768] fp32 in SBUF, updated in place
  - LayerNorm token-major (bn_stats/bn_aggr) -> xhat bf16 -> PE-transpose
    (batched per token tile) to feature-major XHT [128 d, 6, 1024 tok] bf16
  - per head-pair (2 heads x 64 hs = 128 partitions): Q then K projected with
    bf16 matmuls on a dedicated psum tag, evicted bf16
  - scoresT per (k-tile, head): bf16 row-located matmuls (head A partitions
    0-63, head B 64-127, auto row tile position) into ping-ponged psum tiles
    -> exp on ACT (scale=1/8 folded in; no max subtraction - logits are O(1)
    by construction) -> bf16
  - exp@V and the softmax denominator (all-ones lhsT) col-packed per head
    pair into one [128, 1024] psum (tile_position=(0, 64) for head B);
    normalize via DVE reciprocal+mul; PE-transpose back (batched, deferred
    one pair for overlap); residual added into X in place
  - FFN: h1 feature-major bf16 per ff tile on ping-ponged psum, Silu on ACT
    -> H2 bf16 resident; W2 cast to bf16 ahead of time (gpsimd); second
    matmul bf16; residual fused into the psum eviction
  - weights stream from HBM in chunks (f32) and are cast to bf16 on
    gpsimd/DVE off the critical path; fp32 accumulation everywhere in PSUM
"""

import os
import sys

import numpy as np

for _p in ("/opt/trn_rl_repo", "/root/.axon_site/_ro/trn_rl_repo"):
    if os.path.isdir(_p) and _p not in sys.path:
        sys.path.insert(0, _p)

import concourse.bass as bass
import concourse.mybir as mybir
import concourse.tile as tile
from concourse import bacc
from concourse.bass_utils import run_bass_kernel_spmd
from concourse.masks import make_identity

F32 = mybir.dt.float32
F32R = mybir.dt.float32r
BF16 = mybir.dt.bfloat16
F8 = mybir.dt.float8e4
AF = mybir.ActivationFunctionType
OP = mybir.AluOpType
DR = mybir.MatmulPerfMode.DoubleRow

B, T, TC, D, H, HS, FF = 8, 1024, 768 // 3, 768, 12, 64, 3072
P = 128
NT = T // P      # 8 token tiles
NTC = TC // P    # 2 context token tiles
ND = D // P      # 6 feature tiles
NF = FF // P     # 24 ffn tiles
NP = H // 2      # 6 head pairs
EPS = 1e-5
SCALE = HS ** -0.5

WEIGHT_NAMES = [
    "ln1_w", "ln1_b", "sWq", "sbq", "sWk", "sbk", "sWv", "sbv",
    "ln2_w", "ln2_b", "cWq", "cbq", "cWk", "cbk", "cWv", "cbv",
    "ln3_w", "ln3_b", "W1", "b1", "W2", "b2",
]


def _build(flags):
    nc = bacc.Bacc("TRN2", target_bir_lowering=False, debug=False)

    d_img = nc.dram_tensor("img_embedding", [T, D], F32, kind="ExternalInput")
    d_ctx = nc.dram_tensor("context", [TC, D], F32, kind="ExternalInput")
    dw = {}
    for i in (1, 2, 3):
        dw[f"ln{i}_w"] = nc.dram_tensor(f"ln{i}_w", [D], F32, kind="ExternalInput")
        dw[f"ln{i}_b"] = nc.dram_tensor(f"ln{i}_b", [D], F32, kind="ExternalInput")
    for nm in ["sWq", "sWk", "sWv", "cWq", "cWk", "cWv"]:
        dw[nm] = nc.dram_tensor(nm, [H, D, HS], F32, kind="ExternalInput")
    for nm in ["sbq", "sbk", "sbv", "cbq", "cbk", "cbv"]:
        dw[nm] = nc.dram_tensor(nm, [H, HS], F32, kind="ExternalInput")
    dw["W1"] = nc.dram_tensor("W1", [D, FF], F32, kind="ExternalInput")
    dw["b1"] = nc.dram_tensor("b1", [FF], F32, kind="ExternalInput")
    dw["W2"] = nc.dram_tensor("W2", [FF, D], F32, kind="ExternalInput")
    dw["b2"] = nc.dram_tensor("b2", [D], F32, kind="ExternalInput")
    d_out = nc.dram_tensor("out", [T, D], F32, kind="ExternalOutput")
    out_ap = d_out.ap().rearrange("(n p) d -> p n d", p=P)

    with tile.TileContext(nc) as tc, (
        tc.tile_pool(name="const", bufs=1)
    ) as const, (
        tc.tile_pool(name="resid", bufs=1)
    ) as resid, (
        tc.tile_pool(name="wpool", bufs=2)
    ) as wpool, (
        tc.tile_pool(name="big", bufs=1)
    ) as big, (
        tc.tile_pool(name="small", bufs=2)
    ) as small, (
        tc.tile_pool(name="stats", bufs=3)
    ) as stats, (
        tc.tile_pool(name="ps", bufs=1, space="PSUM")
    ) as ps:

        # ---- constants ---------------------------------------------------
        idb = const.tile([P, P], BF16)
        make_identity(nc, idb)
        eps_t = const.tile([P, 1], F32)
        nc.vector.memset(eps_t[:], EPS)

        def bcast_row(dram_ap, n):
            t = const.tile([P, n], F32)
            src = bass.AP(tensor=dram_ap.tensor, offset=dram_ap.offset,
                          ap=[[0, P]] + list(dram_ap.ap))
            nc.gpsimd.dma_start(t[:], src)
            return t

        ln_w_t, ln_b_t = {}, {}
        for i in (1, 2, 3):
            if not flags[f"ln{i}_w_triv"]:
                ln_w_t[i] = bcast_row(dw[f"ln{i}_w"].ap(), D)
            if not flags[f"ln{i}_b_triv"]:
                ln_b_t[i] = bcast_row(dw[f"ln{i}_b"].ap(), D)
        b2_t = None if flags["b2_zero"] else bcast_row(dw["b2"].ap(), D)

        def pair_bias(nm):
            t = const.tile([P, NP], F32)
            nc.sync.dma_start(
                t[:], dw[nm].ap().rearrange("(g i) e -> (i e) g", i=2))
            return t

        sbq_t = None if flags["sbq_zero"] else pair_bias("sbq")
        sbk_t = None if flags["sbk_zero"] else pair_bias("sbk")
        cbq_t = None if flags["cbq_zero"] else pair_bias("cbq")
        cbk_t = None if flags["cbk_zero"] else pair_bias("cbk")
        sbv_t = None if flags["sbv_zero"] else bcast_row(
            dw["sbv"].ap().rearrange("h e -> (h e)"), D)
        cbv_t = None if flags["cbv_zero"] else bcast_row(
            dw["cbv"].ap().rearrange("h e -> (h e)"), D)
        b1_t = None
        if not flags["b1_zero"]:
            b1_t = const.tile([P, NF], F32)
            nc.sync.dma_start(b1_t[:], dw["b1"].ap().rearrange("(f p) -> p f", p=P))

        # ---- residual stream + context (transposed, fp8) ----------------
        ctxT = resid.tile([P, ND, TC], F8)
        for t in range(NTC):
            cbf = small.tile([P, D], BF16, tag="xh")
            nc.gpsimd.dma_start(cbf[:], d_ctx.ap().rearrange(
                "(n p) d -> p n d", p=P)[:, t])
            pt = ps.tile([P, D], BF16, tag=("sA" if t % 2 == 0 else "sB"))
            for j in range(ND):
                nc.tensor.transpose(pt[:, j * P:(j + 1) * P],
                                    cbf[:, j * P:(j + 1) * P], idb[:])
            nc.vector.tensor_copy(ctxT[:, :, t * P:(t + 1) * P], pt[:].rearrange(
                "p (j q) -> p j q", q=P))

        X = resid.tile([P, NT, D], F32)
        img_t = d_img.ap().rearrange("(n p) d -> p n d", p=P)
        for t in range(NT):
            nc.sync.dma_start(X[:, t], img_t[:, t])

        # ---- helpers -----------------------------------------------------
        def load_pair_chunk_f8(nm, g):
            """Two heads (2g, 2g+1) of [H, D, HS] -> fp8 [128, ND, 128],
            cast in-flight by the software-DGE DMA."""
            wb = wpool.tile([P, ND, P], F8, tag="wbf")
            for i in range(2):
                nc.gpsimd.dma_start(
                    wb[:, :, i * HS:(i + 1) * HS],
                    dw[nm].ap()[2 * g + i].rearrange("(dt p) e -> p dt e", p=P))
            return wb

        def layernorm_to_T(i, XHT):
            for t in range(NT):
                st = stats.tile([P, 3, 6], F32, tag="bst")
                xg = X[:, t, :].rearrange("p (g d) -> p g d", d=256)
                for g in range(3):
                    nc.vector.bn_stats(st[:, g, :], xg[:, g, :])
                mv = stats.tile([P, 2], F32, tag="mv")
                nc.vector.bn_aggr(mv[:], st[:])
                sd = stats.tile([P, 1], F32, tag="sd")
                nc.scalar.activation(sd[:], mv[:, 1:2], AF.Sqrt, bias=eps_t[:])
                rstd = stats.tile([P, 1], F32, tag="rstd")
                nc.vector.reciprocal(rstd[:], sd[:])
                nmr = stats.tile([P, 1], F32, tag="nmr")
                nc.vector.tensor_scalar(nmr[:], mv[:, 0:1], rstd[:], -1.0,
                                        OP.mult, OP.mult)
                if i in ln_w_t or i in ln_b_t:
                    xf = small.tile([P, D], F32, tag="fst")
                    nc.vector.tensor_scalar(xf[:], X[:, t, :], mv[:, 0:1],
                                            rstd[:], OP.subtract, OP.mult)
                    xh = small.tile([P, D], BF16, tag="xh")
                    if i in ln_w_t and i in ln_b_t:
                        nc.vector.tensor_mul(xf[:], xf[:], ln_w_t[i][:])
                        nc.vector.tensor_tensor(xh[:], xf[:], ln_b_t[i][:], OP.add)
                    elif i in ln_w_t:
                        nc.vector.tensor_tensor(xh[:], xf[:], ln_w_t[i][:], OP.mult)
                    else:
                        nc.vector.tensor_tensor(xh[:], xf[:], ln_b_t[i][:], OP.add)
                else:
                    xh = small.tile([P, D], BF16, tag="xh")
                    nc.scalar.activation(xh[:], X[:, t, :], AF.Identity,
                                         bias=nmr[:], scale=rstd[:])
                pt = ps.tile([P, D], BF16, tag=("sA" if t % 2 == 0 else "sB"))
                for j in range(ND):
                    nc.tensor.transpose(pt[:, j * P:(j + 1) * P],
                                        xh[:, j * P:(j + 1) * P], idb[:])
                nc.vector.tensor_copy(
                    XHT[:, :, t * P:(t + 1) * P],
                    pt[:].rearrange("p (j q) -> p j q", q=P))

        def project_v(nm, XT, n_tok, dest, bias_t):
            """dest [P tok, n_tok//P, H, HS+1] fp8: per-head V plus a ones
            column (65th) so the AV matmul also produces the softmax
            denominator at output partition 64."""
            nc.vector.memset(dest[:, :, :, HS:HS + 1], 1.0)
            nc.vector.memset(dest[:, :, :, HS + 1:HS + 4], 0.0)
            wv = big.tile([P, ND, D], F8, tag="wv")
            for dt in range(ND):
                nc.gpsimd.dma_start(
                    wv[:, dt, :].rearrange("p (h e) -> p h e", e=HS),
                    dw[nm].ap().rearrange("h (dt p) e -> dt p h e", p=P)[dt])
            for t in range(n_tok // P):
                pv = ps.tile([P, D], F32, tag="sA")
                for o, w in ((0, 512), (512, 256)):
                    for dt2 in range(ND // 2):
                        nc.tensor.matmul(
                            pv[:, o:o + w],
                            XT[:, 2 * dt2:2 * dt2 + 2, t * P:(t + 1) * P],
                            wv[:, 2 * dt2:2 * dt2 + 2, o:o + w],
                            start=(dt2 == 0), stop=(dt2 == ND // 2 - 1),
                            perf_mode=DR)
                dv = dest[:, t, :, 0:HS]
                pvh = pv[:].rearrange("p (h e) -> p h e", e=HS)
                if bias_t is not None:
                    nc.vector.tensor_tensor(
                        dv, pvh, bias_t[:].rearrange("p (h e) -> p h e", e=HS),
                        OP.add)
                else:
                    nc.vector.tensor_copy(dv, pvh)

        def attention(wq_nm, wk_nm, XT, KXT, n_kv, Vt, qb, kb, pre0=None):
            """Full attention pass; adds output into X in place."""
            nk = n_kv // P
            pending = []

            def flush_attn_out(g, ao8):
                # ao8 [65, 2, T] bf16: rows 0-63 unnormalized head output,
                # row 64 softmax denominator. Transpose token-major, then
                # normalize with per-token (per-partition) reciprocals.
                for i in range(2):
                    for c in range(2):
                        att = ps.tile([P, 4, HS + 2], BF16,
                                      tag=("po0" if (2 * i + c) % 2 == 0
                                           else "po1"))
                        for t4 in range(4):
                            t = c * 4 + t4
                            nc.tensor.transpose(
                                att[:, t4, 0:HS + 1],
                                ao8[0:HS + 1, i, t * P:(t + 1) * P],
                                idb[0:HS + 1, 0:HS + 1])
                        rec = stats.tile([P, 4], F32, tag="rec4")
                        nc.vector.reciprocal(rec[:], att[:, :, HS])
                        tmp = small.tile([P, 4, HS], BF16, tag="tmp")
                        for t4 in range(4):
                            nc.vector.tensor_scalar(
                                tmp[:, t4, :], att[:, t4, 0:HS],
                                rec[:, t4:t4 + 1], None, OP.mult)
                        col = g * P + i * HS
                        xv = X[:, c * 4:(c + 1) * 4, col:col + HS]
                        nc.vector.tensor_tensor(xv, tmp[:], xv, OP.add)

            def do_proj(g):
                if g == 0 and pre0 is not None:
                    wqb, wkb = pre0
                else:
                    wqb = load_pair_chunk_f8(wq_nm, g)
                    wkb = load_pair_chunk_f8(wk_nm, g)
                pq = ps.tile([P, T], F32, tag="pq")
                for c in range(2):
                    for dt2 in range(ND // 2):
                        nc.tensor.matmul(
                            pq[:, c * 512:(c + 1) * 512],
                            wqb[:, 2 * dt2:2 * dt2 + 2, :],
                            XT[:, 2 * dt2:2 * dt2 + 2, c * 512:(c + 1) * 512],
                            start=(dt2 == 0), stop=(dt2 == ND // 2 - 1),
                            perf_mode=DR)
                qg = small.tile([P, T], BF16, tag="qg")
                if qb is not None:
                    nc.vector.tensor_scalar(qg[:], pq[:],
                                            qb[:, g:g + 1], None, OP.add)
                else:
                    nc.vector.tensor_copy(qg[:], pq[:])
                pk = ps.tile([P, n_kv], F32, tag="pq")
                for c in range(max(1, n_kv // 512)):
                    w = min(512, n_kv)
                    for dt2 in range(ND // 2):
                        nc.tensor.matmul(
                            pk[:, c * w:(c + 1) * w],
                            wkb[:, 2 * dt2:2 * dt2 + 2, :],
                            KXT[:, 2 * dt2:2 * dt2 + 2, c * w:(c + 1) * w],
                            start=(dt2 == 0), stop=(dt2 == ND // 2 - 1),
                            perf_mode=DR)
                kg = small.tile([P, n_kv], BF16, tag="kg")
                if kb is not None:
                    nc.vector.tensor_scalar(kg[:], pk[:],
                                            kb[:, g:g + 1], None, OP.add)
                else:
                    nc.vector.tensor_copy(kg[:], pk[:])
                return qg, kg

            carry = do_proj(0)
            for g in range(NP):
                qg, kg = carry

                if nk >= 4:
                    bounds = [2, 2, nk - 4]
                else:
                    bounds = [nk, 0, 0]
                tags = ["exphC", "exphB", "exph"]
                offs = [0, bounds[0], bounds[0] + bounds[1]]
                exs = []
                for j in range(3):
                    if bounds[j] > 0:
                        exs.append(big.tile([P, 2, bounds[j], T], F8,
                                            tag=tags[j], name=f"ex{j}_{g}"))
                    else:
                        exs.append(None)

                def exidx(k):
                    j = 0 if k < offs[1] else (1 if k < offs[2] else 2)
                    return j, k - offs[j]
                for k in range(nk):
                    for i in range(2):
                        eh, ek = exidx(k)
                        sc = ps.tile(
                            [P, T], F32,
                            tag=("sA" if (2 * k + i) % 2 == 0 else "sB"))
                        for c in range(2):
                            nc.tensor.matmul(
                                sc[:, c * 512:(c + 1) * 512],
                                kg[i * HS:(i + 1) * HS, k * P:(k + 1) * P],
                                qg[i * HS:(i + 1) * HS, c * 512:(c + 1) * 512],
                                start=True, stop=True)
                        nc.scalar.activation(exs[eh][:, i, ek, :], sc[:],
                                             AF.Exp, scale=SCALE)

                if g + 1 < NP:
                    carry = do_proj(g + 1)
                while len(pending) > 0:
                    flush_attn_out(*pending.pop(0))

                ao8 = small.tile([P, 2, T], BF16, tag="ao8")
                for c in range(2):
                    for i in range(2):
                        po = ps.tile([P, 512], F32,
                                     tag=("po0" if (2 * c + i) % 2 == 0
                                          else "po1"))
                        for kp in range(nk // 2):
                            eh, ek = exidx(2 * kp)
                            nc.tensor.matmul(
                                po[0:HS + 4, :],
                                Vt[:, 2 * kp:2 * kp + 2, 2 * g + i, 0:HS + 4],
                                exs[eh][:, i, ek:ek + 2,
                                        c * 512:(c + 1) * 512],
                                start=(kp == 0), stop=(kp == nk // 2 - 1),
                                perf_mode=DR)
                        nc.vector.tensor_copy(
                            ao8[0:HS + 1, i, c * 512:(c + 1) * 512],
                            po[0:HS + 1, :])

                pending.append((g, ao8))

            while pending:
                flush_attn_out(*pending.pop(0))

        # =================== self attention ==============================
        XHT = big.tile([P, ND, T], F8, tag="xht")
        pre0 = (load_pair_chunk_f8("sWq", 0), load_pair_chunk_f8("sWk", 0))
        layernorm_to_T(1, XHT)
        V = big.tile([P, NT, H, HS + 4], F8, tag="vw2")
        project_v("sWv", XHT, T, V, sbv_t)
        attention("sWq", "sWk", XHT, XHT, T, V, sbq_t, sbk_t, pre0=pre0)

        # =================== cross attention =============================
        XHT2 = big.tile([P, ND, T], F8, tag="xht")
        layernorm_to_T(2, XHT2)
        Vc = big.tile([P, NTC, H, HS + 4], F8, tag="vw2")
        project_v("cWv", ctxT, TC, Vc, cbv_t)
        attention("cWq", "cWk", XHT2, ctxT, TC, Vc, cbq_t, cbk_t)

        # =================== FFN =========================================
        XHT3 = big.tile([P, ND, T], F8, tag="xht")
        layernorm_to_T(3, XHT3)

        W2b = big.tile([P, NF, D], BF16, tag="vw2")
        for f in range(NF):
            nc.gpsimd.dma_start(W2b[:, f, :], dw["W2"].ap()[f * P:(f + 1) * P, :])

        H2 = big.tile([P, NF, T], BF16, tag="exph")
        for f in range(NF):
            w1b = wpool.tile([P, ND, P], F8, tag="wbf2")
            nc.gpsimd.dma_start(
                w1b[:], dw["W1"].ap()[:, f * P:(f + 1) * P].rearrange(
                    "(dt p) c -> p dt c", p=P))
            ph = ps.tile([P, T], F32, tag=("sA" if f % 2 == 0 else "pq"))
            for c in range(2):
                for dt2 in range(ND // 2):
                    nc.tensor.matmul(
                        ph[:, c * 512:(c + 1) * 512],
                        w1b[:, 2 * dt2:2 * dt2 + 2, :],
                        XHT3[:, 2 * dt2:2 * dt2 + 2, c * 512:(c + 1) * 512],
                        start=(dt2 == 0), stop=(dt2 == ND // 2 - 1),
                        perf_mode=DR)
            nc.scalar.activation(
                H2[:, f, :], ph[:], AF.Silu,
                bias=(b1_t[:, f:f + 1] if b1_t is not None else 0.0))


        for t in range(NT):
            pf = ps.tile([P, D], F32, tag=("sB" if t % 2 == 0 else "pq"))
            for o, w in ((0, 512), (512, 256)):
                for f in range(NF):
                    nc.tensor.matmul(
                        pf[:, o:o + w],
                        H2[:, f, t * P:(t + 1) * P],
                        W2b[:, f, o:o + w],
                        start=(f == 0), stop=(f == NF - 1))
            ot = small.tile([P, D], F32, tag="ot")
            nc.vector.tensor_tensor(ot[:], pf[:], X[:, t, :], OP.add)
            if b2_t is not None:
                nc.vector.tensor_add(ot[:], ot[:], b2_t[:])
            nc.sync.dma_start(out_ap[:, t], ot[:])

    nc.compile()
    return nc


_CACHE = {}


def _flags_of(inputs):
    f = {}
    for i in (1, 2, 3):
        f[f"ln{i}_w_triv"] = bool(np.all(inputs[f"ln{i}_w"] == 1.0))
        f[f"ln{i}_b_triv"] = bool(np.all(inputs[f"ln{i}_b"] == 0.0))
    for nm in ["sbq", "sbk", "sbv", "cbq", "cbk", "cbv", "b1", "b2"]:
        f[f"{nm}_zero"] = bool(np.all(inputs[nm] == 0.0))
    return f


def kernel(**inputs):
    flags = _flags_of(inputs)
    key = tuple(sorted(flags.items()))
    if key not in _CACHE:
        _CACHE[key] = _build(flags)
    nc = _CACHE[key]

    in_maps = []
    for b in range(B):
        m = {"img_embedding": np.ascontiguousarray(
                 inputs["img_embedding"][b].astype(np.float32)),
             "context": np.ascontiguousarray(
                 inputs["context"][b].astype(np.float32))}
        for nm in WEIGHT_NAMES:
            m[nm] = np.ascontiguousarray(inputs[nm].astype(np.float32))
        in_maps.append(m)

    res = run_bass_kernel_spmd(nc, in_maps, core_ids=list(range(B)))
    return np.stack([res.results[b]["out"] for b in range(B)], axis=0)



# revision 38
# speedup vs baseline: 1.2738x; 1.0562x over previous
"""DiT block kernel for Trainium2 (Bass/Tile), data-parallel over batch on 8 cores.

Per-core dataflow (one batch element per core; no collectives needed):
  - residual stream X [128 tok, 8, 768] fp32 in SBUF, updated in place
  - LayerNorm token-major (bn_stats/bn_aggr) -> xhat bf16 -> PE-transpose
    (batched per token tile) to feature-major XHT [128 d, 6, 1024 tok] bf16
  - per head-pair (2 heads x 64 hs = 128 partitions): Q then K projected with
    bf16 matmuls on a dedicated psum tag, evicted bf16
  - scoresT per (k-tile, head): bf16 row-located matmuls (head A partitions
    0-63, head B 64-127, auto row tile position) into ping-ponged psum tiles
    -> exp on ACT (scale=1/8 folded in; no max subtraction - logits are O(1)
    by construction) -> bf16
  - exp@V and the softmax denominator (all-ones lhsT) col-packed per head
    pair into one [128, 1024] psum (tile_position=(0, 64) for head B);
    normalize via DVE reciprocal+mul; PE-transpose back (batched, deferred
    one pair for overlap); residual added into X in place
  - FFN: h1 feature-major bf16 per ff tile on ping-ponged psum, Silu on ACT
    -> H2 bf16 resident; W2 cast to bf16 ahead of time (gpsimd); second
    matmul bf16; residual fused into the psum eviction
  - weights stream from HBM in chunks (f32) and are cast to bf16 on
    gpsimd/DVE off the critical path; fp32 accumulation everywhere in PSUM
"""

import os
import sys

import numpy as np

for _p in ("/opt/trn_rl_repo", "/root/.axon_site/_ro/trn_rl_repo"):
    if os.path.isdir(_p) and _p not in sys.path:
        sys.path.insert(0, _p)

import concourse.bass as bass
import concourse.mybir as mybir
import concourse.tile as tile
from concourse import bacc
from concourse.bass_utils import run_bass_kernel_spmd
from concourse.masks import make_identity

F32 = mybir.dt.float32
F32R = mybir.dt.float32r
BF16 = mybir.dt.bfloat16
F8 = mybir.dt.float8e4
AF = mybir.ActivationFunctionType
OP = mybir.AluOpType
DR = mybir.MatmulPerfMode.DoubleRow

B, T, TC, D, H, HS, FF = 8, 1024, 768 // 3, 768, 12, 64, 3072
P = 128
NT = T // P      # 8 token tiles
NTC = TC // P    # 2 context token tiles
ND = D // P      # 6 feature tiles
NF = FF // P     # 24 ffn tiles
NP = H // 2      # 6 head pairs
EPS = 1e-5
SCALE = HS ** -0.5

WEIGHT_NAMES = [
    "ln1_w", "ln1_b", "sWq", "sbq", "sWk", "sbk", "sWv", "sbv",
    "ln2_w", "ln2_b", "cWq", "cbq", "cWk", "cbk", "cWv", "cbv",
    "ln3_w", "ln3_b", "W1", "b1", "W2", "b2",
]


def _build(flags):
    nc = bacc.Bacc("TRN2", target_bir_lowering=False, debug=False)

    d_img = nc.dram_tensor("img_embedding", [T, D], F32, kind="ExternalInput")
    d_ctx = nc.dram_tensor("context", [TC, D], F32, kind="ExternalInput")
    dw = {}
    for i in (1, 2, 3):
        dw[f"ln{i}_w"] = nc.dram_tensor(f"ln{i}_w", [D], F32, kind="ExternalInput")
        dw[f"ln{i}_b"] = nc.dram_tensor(f"ln{i}_b", [D], F32, kind="ExternalInput")
    for nm in ["sWq", "sWk", "sWv", "cWq", "cWk", "cWv"]:
        dw[nm] = nc.dram_tensor(nm, [H, D, HS], F32, kind="ExternalInput")
    for nm in ["sbq", "sbk", "sbv", "cbq", "cbk", "cbv"]:
        dw[nm] = nc.dram_tensor(nm, [H, HS], F32, kind="ExternalInput")
    dw["W1"] = nc.dram_tensor("W1", [D, FF], F32, kind="ExternalInput")
    dw["b1"] = nc.dram_tensor("b1", [FF], F32, kind="ExternalInput")
    dw["W2"] = nc.dram_tensor("W2", [FF, D], F32, kind="ExternalInput")
    dw["b2"] = nc.dram_tensor("b2", [D], F32, kind="ExternalInput")
    d_out = nc.dram_tensor("out", [T, D], F32, kind="ExternalOutput")
    out_ap = d_out.ap().rearrange("(n p) d -> p n d", p=P)

    with tile.TileContext(nc) as tc, (
        tc.tile_pool(name="const", bufs=1)
    ) as const, (
        tc.tile_pool(name="resid", bufs=1)
    ) as resid, (
        tc.tile_pool(name="wpool", bufs=2)
    ) as wpool, (
        tc.tile_pool(name="big", bufs=1)
    ) as big, (
        tc.tile_pool(name="small", bufs=2)
    ) as small, (
        tc.tile_pool(name="stats", bufs=3)
    ) as stats, (
        tc.tile_pool(name="ps", bufs=1, space="PSUM")
    ) as ps:

        # ---- constants ---------------------------------------------------
        idb = const.tile([P, P], BF16)
        make_identity(nc, idb)
        eps_t = const.tile([P, 1], F32)
        nc.vector.memset(eps_t[:], EPS)

        def bcast_row(dram_ap, n):
            t = const.tile([P, n], F32)
            src = bass.AP(tensor=dram_ap.tensor, offset=dram_ap.offset,
                          ap=[[0, P]] + list(dram_ap.ap))
            nc.gpsimd.dma_start(t[:], src)
            return t

        ln_w_t, ln_b_t = {}, {}
        for i in (1, 2, 3):
            if not flags[f"ln{i}_w_triv"]:
                ln_w_t[i] = bcast_row(dw[f"ln{i}_w"].ap(), D)
            if not flags[f"ln{i}_b_triv"]:
                ln_b_t[i] = bcast_row(dw[f"ln{i}_b"].ap(), D)
        b2_t = None if flags["b2_zero"] else bcast_row(dw["b2"].ap(), D)

        def pair_bias(nm):
            t = const.tile([P, NP], F32)
            nc.sync.dma_start(
                t[:], dw[nm].ap().rearrange("(g i) e -> (i e) g", i=2))
            return t

        sbq_t = None if flags["sbq_zero"] else pair_bias("sbq")
        sbk_t = None if flags["sbk_zero"] else pair_bias("sbk")
        cbq_t = None if flags["cbq_zero"] else pair_bias("cbq")
        cbk_t = None if flags["cbk_zero"] else pair_bias("cbk")
        sbv_t = None if flags["sbv_zero"] else bcast_row(
            dw["sbv"].ap().rearrange("h e -> (h e)"), D)
        cbv_t = None if flags["cbv_zero"] else bcast_row(
            dw["cbv"].ap().rearrange("h e -> (h e)"), D)
        b1_t = None
        if not flags["b1_zero"]:
            b1_t = const.tile([P, NF], F32)
            nc.sync.dma_start(b1_t[:], dw["b1"].ap().rearrange("(f p) -> p f", p=P))

        # ---- residual stream + context (transposed, fp8) ----------------
        ctxT = resid.tile([P, ND, TC], F8)
        for t in range(NTC):
            cbf = small.tile([P, D], BF16, tag="xh")
            nc.gpsimd.dma_start(cbf[:], d_ctx.ap().rearrange(
                "(n p) d -> p n d", p=P)[:, t])
            pt = ps.tile([P, D], BF16, tag=("sA" if t % 2 == 0 else "sB"))
            for j in range(ND):
                nc.tensor.transpose(pt[:, j * P:(j + 1) * P],
                                    cbf[:, j * P:(j + 1) * P], idb[:])
            nc.vector.tensor_copy(ctxT[:, :, t * P:(t + 1) * P], pt[:].rearrange(
                "p (j q) -> p j q", q=P))

        X = resid.tile([P, NT, D], F32)
        img_t = d_img.ap().rearrange("(n p) d -> p n d", p=P)
        for t in range(NT):
            nc.sync.dma_start(X[:, t], img_t[:, t])

        # ---- helpers -----------------------------------------------------
        def load_whd(nm, tag):
            """Full [H, D, HS] weight -> fp8 [128 dpart, ND, H*HS], cast
            in-flight by the software-DGE DMA (one DMA per d-tile)."""
            w8 = big.tile([P, ND, D], F8, tag=tag)
            for dt in range(ND):
                nc.gpsimd.dma_start(
                    w8[:, dt, :].rearrange("p (h e) -> p h e", e=HS),
                    dw[nm].ap().rearrange("h (dt p) e -> dt p h e", p=P)[dt])
            return w8

        def layernorm_to_T(i, XHT):
            for t in range(NT):
                st = stats.tile([P, 3, 6], F32, tag="bst")
                xg = X[:, t, :].rearrange("p (g d) -> p g d", d=256)
                for g in range(3):
                    nc.vector.bn_stats(st[:, g, :], xg[:, g, :])
                mv = stats.tile([P, 2], F32, tag="mv")
                nc.vector.bn_aggr(mv[:], st[:])
                sd = stats.tile([P, 1], F32, tag="sd")
                nc.scalar.activation(sd[:], mv[:, 1:2], AF.Sqrt, bias=eps_t[:])
                rstd = stats.tile([P, 1], F32, tag="rstd")
                nc.vector.reciprocal(rstd[:], sd[:])
                nmr = stats.tile([P, 1], F32, tag="nmr")
                nc.vector.tensor_scalar(nmr[:], mv[:, 0:1], rstd[:], -1.0,
                                        OP.mult, OP.mult)
                if i in ln_w_t or i in ln_b_t:
                    xf = small.tile([P, D], F32, tag="fst")
                    nc.vector.tensor_scalar(xf[:], X[:, t, :], mv[:, 0:1],
                                            rstd[:], OP.subtract, OP.mult)
                    xh = small.tile([P, D], BF16, tag="xh")
                    if i in ln_w_t and i in ln_b_t:
                        nc.vector.tensor_mul(xf[:], xf[:], ln_w_t[i][:])
                        nc.vector.tensor_tensor(xh[:], xf[:], ln_b_t[i][:], OP.add)
                    elif i in ln_w_t:
                        nc.vector.tensor_tensor(xh[:], xf[:], ln_w_t[i][:], OP.mult)
                    else:
                        nc.vector.tensor_tensor(xh[:], xf[:], ln_b_t[i][:], OP.add)
                else:
                    xh = small.tile([P, D], BF16, tag="xh")
                    nc.scalar.activation(xh[:], X[:, t, :], AF.Identity,
                                         bias=nmr[:], scale=rstd[:])
                pt = ps.tile([P, D], BF16, tag=("sA" if t % 2 == 0 else "sB"))
                for j in range(ND):
                    nc.tensor.transpose(pt[:, j * P:(j + 1) * P],
                                        xh[:, j * P:(j + 1) * P], idb[:])
                nc.vector.tensor_copy(
                    XHT[:, :, t * P:(t + 1) * P],
                    pt[:].rearrange("p (j q) -> p j q", q=P))

        def project_v(wv, XT, n_tok, dest, bias_t):
            """dest [P tok, n_tok//P, H, HS+4] fp8: per-head V plus a ones
            column (65th) so the AV matmul also produces the softmax
            denominator at output partition 64."""
            nc.vector.memset(dest[:, :, :, HS:HS + 1], 1.0)
            nc.vector.memset(dest[:, :, :, HS + 1:HS + 4], 0.0)
            for t in range(n_tok // P):
                pv = ps.tile([P, D], F32, tag="sA")
                for o, w in ((0, 512), (512, 256)):
                    for dt2 in range(ND // 2):
                        nc.tensor.matmul(
                            pv[:, o:o + w],
                            XT[:, 2 * dt2:2 * dt2 + 2, t * P:(t + 1) * P],
                            wv[:, 2 * dt2:2 * dt2 + 2, o:o + w],
                            start=(dt2 == 0), stop=(dt2 == ND // 2 - 1),
                            perf_mode=DR)
                dv = dest[:, t, :, 0:HS]
                pvh = pv[:].rearrange("p (h e) -> p h e", e=HS)
                if bias_t is not None:
                    nc.vector.tensor_tensor(
                        dv, pvh, bias_t[:].rearrange("p (h e) -> p h e", e=HS),
                        OP.add)
                else:
                    nc.vector.tensor_copy(dv, pvh)

        def attention(wq8, wk8, XT, KXT, n_kv, Vt, qb, kb, use_act=False):
            """Full attention pass; adds output into X in place."""
            nk = n_kv // P
            pending = []

            def flush_attn_out(g, ao8):
                # ao8 [65, 2, T] bf16: rows 0-63 unnormalized head output,
                # row 64 softmax denominator. Transpose token-major, then
                # normalize with per-token (per-partition) reciprocals.
                for i in range(2):
                    for c in range(2):
                        att = ps.tile([P, 4, HS + 2], BF16,
                                      tag=("po0" if (2 * i + c) % 2 == 0
                                           else "po1"))
                        for t4 in range(4):
                            t = c * 4 + t4
                            nc.tensor.transpose(
                                att[:, t4, 0:HS + 1],
                                ao8[0:HS + 1, i, t * P:(t + 1) * P],
                                idb[0:HS + 1, 0:HS + 1])
                        rec = stats.tile([P, 4], F32, tag="rec4")
                        nc.vector.reciprocal(rec[:], att[:, :, HS])
                        tmp = small.tile([P, 4, HS], BF16, tag="tmp")
                        for t4 in range(4):
                            if use_act and i == 1:
                                nc.scalar.mul(tmp[:, t4, :],
                                              att[:, t4, 0:HS],
                                              rec[:, t4:t4 + 1])
                            else:
                                nc.vector.tensor_scalar(
                                    tmp[:, t4, :], att[:, t4, 0:HS],
                                    rec[:, t4:t4 + 1], None, OP.mult)
                        col = g * P + i * HS
                        xv = X[:, c * 4:(c + 1) * 4, col:col + HS]
                        nc.vector.tensor_tensor(xv, tmp[:], xv, OP.add)

            def do_proj(g):
                pq = ps.tile([P, T], F32, tag="pq")
                for c in range(2):
                    for dt2 in range(ND // 2):
                        nc.tensor.matmul(
                            pq[:, c * 512:(c + 1) * 512],
                            wq8[:, 2 * dt2:2 * dt2 + 2, g * P:(g + 1) * P],
                            XT[:, 2 * dt2:2 * dt2 + 2, c * 512:(c + 1) * 512],
                            start=(dt2 == 0), stop=(dt2 == ND // 2 - 1),
                            perf_mode=DR)
                qg = small.tile([P, T], BF16, tag="qg")
                if qb is not None:
                    nc.vector.tensor_scalar(qg[:], pq[:],
                                            qb[:, g:g + 1], None, OP.add)
                elif use_act:
                    nc.scalar.copy(qg[:], pq[:])
                else:
                    nc.vector.tensor_copy(qg[:], pq[:])
                pk = ps.tile([P, n_kv], F32, tag="pq")
                for c in range(max(1, n_kv // 512)):
                    w = min(512, n_kv)
                    for dt2 in range(ND // 2):
                        nc.tensor.matmul(
                            pk[:, c * w:(c + 1) * w],
                            wk8[:, 2 * dt2:2 * dt2 + 2, g * P:(g + 1) * P],
                            KXT[:, 2 * dt2:2 * dt2 + 2, c * w:(c + 1) * w],
                            start=(dt2 == 0), stop=(dt2 == ND // 2 - 1),
                            perf_mode=DR)
                kg = small.tile([P, n_kv], BF16, tag="kg")
                if kb is not None:
                    nc.vector.tensor_scalar(kg[:], pk[:],
                                            kb[:, g:g + 1], None, OP.add)
                elif use_act:
                    nc.scalar.copy(kg[:], pk[:])
                else:
                    nc.vector.tensor_copy(kg[:], pk[:])
                return qg, kg

            carry = do_proj(0)
            for g in range(NP):
                qg, kg = carry

                if nk >= 4:
                    bounds = [2, 2, nk - 4]
                else:
                    bounds = [nk, 0, 0]
                tags = ["exphC", "exphB", "exph"]
                offs = [0, bounds[0], bounds[0] + bounds[1]]
                exs = []
                for j in range(3):
                    if bounds[j] > 0:
                        exs.append(big.tile([P, 2, bounds[j], T], F8,
                                            tag=tags[j], name=f"ex{j}_{g}"))
                    else:
                        exs.append(None)

                def exidx(k):
                    j = 0 if k < offs[1] else (1 if k < offs[2] else 2)
                    return j, k - offs[j]
                for k in range(nk):
                    for i in range(2):
                        eh, ek = exidx(k)
                        sc = ps.tile(
                            [P, T], F32,
                            tag=("sA" if (2 * k + i) % 2 == 0 else "sB"))
                        for c in range(2):
                            nc.tensor.matmul(
                                sc[:, c * 512:(c + 1) * 512],
                                kg[i * HS:(i + 1) * HS, k * P:(k + 1) * P],
                                qg[i * HS:(i + 1) * HS, c * 512:(c + 1) * 512],
                                start=True, stop=True)
                        nc.scalar.activation(exs[eh][:, i, ek, :], sc[:],
                                             AF.Exp, scale=SCALE)

                if g + 1 < NP:
                    carry = do_proj(g + 1)
                while len(pending) > 0:
                    flush_attn_out(*pending.pop(0))

                ao8 = small.tile([P, 2, T], BF16, tag="ao8")
                for c in range(2):
                    for i in range(2):
                        po = ps.tile([P, 512], F32,
                                     tag=("po0" if (2 * c + i) % 2 == 0
                                          else "po1"))
                        for kp in range(nk // 2):
                            eh, ek = exidx(2 * kp)
                            nc.tensor.matmul(
                                po[0:HS + 4, :],
                                Vt[:, 2 * kp:2 * kp + 2, 2 * g + i, 0:HS + 4],
                                exs[eh][:, i, ek:ek + 2,
                                        c * 512:(c + 1) * 512],
                                start=(kp == 0), stop=(kp == nk // 2 - 1),
                                perf_mode=DR)
                        if use_act and c == 1:
                            nc.scalar.copy(
                                ao8[0:HS + 1, i, c * 512:(c + 1) * 512],
                                po[0:HS + 1, :])
                        else:
                            nc.vector.tensor_copy(
                                ao8[0:HS + 1, i, c * 512:(c + 1) * 512],
                                po[0:HS + 1, :])

                pending.append((g, ao8))

            while pending:
                flush_attn_out(*pending.pop(0))

        # =================== weight preloads (issue order = need order) ==
        wq_s = load_whd("sWq", "wqs")
        wk_s = load_whd("sWk", "wks")
        wv_s = load_whd("sWv", "wvs")
        wq_c = load_whd("cWq", "wqc")
        wk_c = load_whd("cWk", "wkc")
        wv_c = load_whd("cWv", "wvc")
        w1f = big.tile([P, ND, FF], F8, tag="w1")
        nc.gpsimd.dma_start(
            w1f[:], dw["W1"].ap().rearrange("(dt p) c -> p dt c", p=P))
        W2b = big.tile([P, NF, D], BF16, tag="w2")
        nc.gpsimd.dma_start(
            W2b[:], dw["W2"].ap().rearrange("(f p) d -> p f d", p=P))

        # =================== self attention ==============================
        XHT = big.tile([P, ND, T], F8, tag="xht")
        layernorm_to_T(1, XHT)
        V = big.tile([P, NT, H, HS + 4], F8, tag="vw2")
        project_v(wv_s, XHT, T, V, sbv_t)
        attention(wq_s, wk_s, XHT, XHT, T, V, sbq_t, sbk_t)

        # =================== cross attention =============================
        XHT2 = big.tile([P, ND, T], F8, tag="xht")
        layernorm_to_T(2, XHT2)
        Vc = big.tile([P, NTC, H, HS + 4], F8, tag="vw2")
        project_v(wv_c, ctxT, TC, Vc, cbv_t)
        attention(wq_c, wk_c, XHT2, ctxT, TC, Vc, cbq_t, cbk_t, use_act=True)

        # =================== FFN =========================================
        XHT3 = big.tile([P, ND, T], F8, tag="xht")
        layernorm_to_T(3, XHT3)

        H2 = big.tile([P, NF, T], BF16, tag="exph")
        for f in range(NF):
            ph = ps.tile([P, T], F32, tag=("sA" if f % 2 == 0 else "pq"))
            for c in range(2):
                for dt2 in range(ND // 2):
                    nc.tensor.matmul(
                        ph[:, c * 512:(c + 1) * 512],
                        w1f[:, 2 * dt2:2 * dt2 + 2, f * P:(f + 1) * P],
                        XHT3[:, 2 * dt2:2 * dt2 + 2, c * 512:(c + 1) * 512],
                        start=(dt2 == 0), stop=(dt2 == ND // 2 - 1),
                        perf_mode=DR)
            nc.scalar.activation(
                H2[:, f, :], ph[:], AF.Silu,
                bias=(b1_t[:, f:f + 1] if b1_t is not None else 0.0))


        for t in range(NT):
            pf = ps.tile([P, D], F32, tag=("sB" if t % 2 == 0 else "pq"))
            for o, w in ((0, 512), (512, 256)):
                for f in range(NF):
                    nc.tensor.matmul(
                        pf[:, o:o + w],
                        H2[:, f, t * P:(t + 1) * P],
                        W2b[:, f, o:o + w],
                        start=(f == 0), stop=(f == NF - 1))
            ot = small.tile([P, D], F32, tag="ot")
            nc.vector.tensor_tensor(ot[:], pf[:], X[:, t, :], OP.add)
            if b2_t is not None:
                nc.vector.tensor_add(ot[:], ot[:], b2_t[:])
            nc.sync.dma_start(out_ap[:, t], ot[:])

    nc.compile()
    return nc


_CACHE = {}


def _flags_of(inputs):
    f = {}
    for i in (1, 2, 3):
        f[f"ln{i}_w_triv"] = bool(np.all(inputs[f"ln{i}_w"] == 1.0))
        f[f"ln{i}_b_triv"] = bool(np.all(inputs[f"ln{i}_b"] == 0.0))
    for nm in ["sbq", "sbk", "sbv", "cbq", "cbk", "cbv", "b1", "b2"]:
        f[f"{nm}_zero"] = bool(np.all(inputs[nm] == 0.0))
    return f


def kernel(**inputs):
    flags = _flags_of(inputs)
    key = tuple(sorted(flags.items()))
    if key not in _CACHE:
        _CACHE[key] = _build(flags)
    nc = _CACHE[key]

    in_maps = []
    for b in range(B):
        m = {"img_embedding": np.ascontiguousarray(
                 inputs["img_embedding"][b].astype(np.float32)),
             "context": np.ascontiguousarray(
                 inputs["context"][b].astype(np.float32))}
        for nm in WEIGHT_NAMES:
            m[nm] = np.ascontiguousarray(inputs[nm].astype(np.float32))
        in_maps.append(m)

    res = run_bass_kernel_spmd(nc, in_maps, core_ids=list(range(B)))
    return np.stack([res.results[b]["out"] for b in range(B)], axis=0)



# revision 40
# speedup vs baseline: 1.2957x; 1.0172x over previous
"""DiT block kernel for Trainium2 (Bass/Tile), data-parallel over batch on 8 cores.

Per-core dataflow (one batch element per core; no collectives needed):
  - residual stream X [128 tok, 8, 768] fp32 in SBUF, updated in place
  - LayerNorm token-major (bn_stats/bn_aggr) -> xhat bf16 -> PE-transpose
    (batched per token tile) to feature-major XHT [128 d, 6, 1024 tok] bf16
  - per head-pair (2 heads x 64 hs = 128 partitions): Q then K projected with
    bf16 matmuls on a dedicated psum tag, evicted bf16
  - scoresT per (k-tile, head): bf16 row-located matmuls (head A partitions
    0-63, head B 64-127, auto row tile position) into ping-ponged psum tiles
    -> exp on ACT (scale=1/8 folded in; no max subtraction - logits are O(1)
    by construction) -> bf16
  - exp@V and the softmax denominator (all-ones lhsT) col-packed per head
    pair into one [128, 1024] psum (tile_position=(0, 64) for head B);
    normalize via DVE reciprocal+mul; PE-transpose back (batched, deferred
    one pair for overlap); residual added into X in place
  - FFN: h1 feature-major bf16 per ff tile on ping-ponged psum, Silu on ACT
    -> H2 bf16 resident; W2 cast to bf16 ahead of time (gpsimd); second
    matmul bf16; residual fused into the psum eviction
  - weights stream from HBM in chunks (f32) and are cast to bf16 on
    gpsimd/DVE off the critical path; fp32 accumulation everywhere in PSUM
"""

import os
import sys

import numpy as np

for _p in ("/opt/trn_rl_repo", "/root/.axon_site/_ro/trn_rl_repo"):
    if os.path.isdir(_p) and _p not in sys.path:
        sys.path.insert(0, _p)

import concourse.bass as bass
import concourse.mybir as mybir
import concourse.tile as tile
from concourse import bacc
from concourse.bass_utils import run_bass_kernel_spmd
from concourse.masks import make_identity

F32 = mybir.dt.float32
F32R = mybir.dt.float32r
BF16 = mybir.dt.bfloat16
F8 = mybir.dt.float8e4
AF = mybir.ActivationFunctionType
OP = mybir.AluOpType
DR = mybir.MatmulPerfMode.DoubleRow

B, T, TC, D, H, HS, FF = 8, 1024, 768 // 3, 768, 12, 64, 3072
P = 128
NT = T // P      # 8 token tiles
NTC = TC // P    # 2 context token tiles
ND = D // P      # 6 feature tiles
NF = FF // P     # 24 ffn tiles
NP = H // 2      # 6 head pairs
EPS = 1e-5
SCALE = HS ** -0.5

WEIGHT_NAMES = [
    "ln1_w", "ln1_b", "sWq", "sbq", "sWk", "sbk", "sWv", "sbv",
    "ln2_w", "ln2_b", "cWq", "cbq", "cWk", "cbk", "cWv", "cbv",
    "ln3_w", "ln3_b", "W1", "b1", "W2", "b2",
]


def _build(flags):
    nc = bacc.Bacc("TRN2", target_bir_lowering=False, debug=False)

    d_img = nc.dram_tensor("img_embedding", [T, D], F32, kind="ExternalInput")
    d_ctx = nc.dram_tensor("context", [TC, D], F32, kind="ExternalInput")
    dw = {}
    for i in (1, 2, 3):
        dw[f"ln{i}_w"] = nc.dram_tensor(f"ln{i}_w", [D], F32, kind="ExternalInput")
        dw[f"ln{i}_b"] = nc.dram_tensor(f"ln{i}_b", [D], F32, kind="ExternalInput")
    for nm in ["sWq", "sWk", "sWv", "cWq", "cWk", "cWv"]:
        dw[nm] = nc.dram_tensor(nm, [H, D, HS], F32, kind="ExternalInput")
    for nm in ["sbq", "sbk", "sbv", "cbq", "cbk", "cbv"]:
        dw[nm] = nc.dram_tensor(nm, [H, HS], F32, kind="ExternalInput")
    dw["W1"] = nc.dram_tensor("W1", [D, FF], F32, kind="ExternalInput")
    dw["b1"] = nc.dram_tensor("b1", [FF], F32, kind="ExternalInput")
    dw["W2"] = nc.dram_tensor("W2", [FF, D], F32, kind="ExternalInput")
    dw["b2"] = nc.dram_tensor("b2", [D], F32, kind="ExternalInput")
    d_out = nc.dram_tensor("out", [T, D], F32, kind="ExternalOutput")
    out_ap = d_out.ap().rearrange("(n p) d -> p n d", p=P)

    with tile.TileContext(nc) as tc, (
        tc.tile_pool(name="const", bufs=1)
    ) as const, (
        tc.tile_pool(name="resid", bufs=1)
    ) as resid, (
        tc.tile_pool(name="wpool", bufs=2)
    ) as wpool, (
        tc.tile_pool(name="big", bufs=1)
    ) as big, (
        tc.tile_pool(name="small", bufs=2)
    ) as small, (
        tc.tile_pool(name="stats", bufs=3)
    ) as stats, (
        tc.tile_pool(name="ps", bufs=1, space="PSUM")
    ) as ps:

        # ---- constants ---------------------------------------------------
        idb = const.tile([P, P], BF16)
        make_identity(nc, idb)
        eps_t = const.tile([P, 1], F32)
        nc.vector.memset(eps_t[:], EPS)

        def bcast_row(dram_ap, n):
            t = const.tile([P, n], F32)
            src = bass.AP(tensor=dram_ap.tensor, offset=dram_ap.offset,
                          ap=[[0, P]] + list(dram_ap.ap))
            nc.gpsimd.dma_start(t[:], src)
            return t

        ln_w_t, ln_b_t = {}, {}
        for i in (1, 2, 3):
            if not flags[f"ln{i}_w_triv"]:
                ln_w_t[i] = bcast_row(dw[f"ln{i}_w"].ap(), D)
            if not flags[f"ln{i}_b_triv"]:
                ln_b_t[i] = bcast_row(dw[f"ln{i}_b"].ap(), D)
        b2_t = None if flags["b2_zero"] else bcast_row(dw["b2"].ap(), D)

        def pair_bias(nm):
            t = const.tile([P, NP], F32)
            nc.sync.dma_start(
                t[:], dw[nm].ap().rearrange("(g i) e -> (i e) g", i=2))
            return t

        sbq_t = None if flags["sbq_zero"] else pair_bias("sbq")
        sbk_t = None if flags["sbk_zero"] else pair_bias("sbk")
        cbq_t = None if flags["cbq_zero"] else pair_bias("cbq")
        cbk_t = None if flags["cbk_zero"] else pair_bias("cbk")
        sbv_t = None if flags["sbv_zero"] else bcast_row(
            dw["sbv"].ap().rearrange("h e -> (h e)"), D)
        cbv_t = None if flags["cbv_zero"] else bcast_row(
            dw["cbv"].ap().rearrange("h e -> (h e)"), D)
        b1_t = None
        if not flags["b1_zero"]:
            b1_t = const.tile([P, NF], F32)
            nc.sync.dma_start(b1_t[:], dw["b1"].ap().rearrange("(f p) -> p f", p=P))

        # ---- residual stream + context (transposed, fp8) ----------------
        ctxT = resid.tile([P, ND, TC], F8)
        for t in range(NTC):
            cbf = small.tile([P, D], BF16, tag="xh")
            nc.gpsimd.dma_start(cbf[:], d_ctx.ap().rearrange(
                "(n p) d -> p n d", p=P)[:, t])
            pt = ps.tile([P, D], BF16, tag=("sA" if t % 2 == 0 else "sB"))
            for j in range(ND):
                nc.tensor.transpose(pt[:, j * P:(j + 1) * P],
                                    cbf[:, j * P:(j + 1) * P], idb[:])
            nc.vector.tensor_copy(ctxT[:, :, t * P:(t + 1) * P], pt[:].rearrange(
                "p (j q) -> p j q", q=P))

        X = resid.tile([P, NT, D], F32)
        img_t = d_img.ap().rearrange("(n p) d -> p n d", p=P)
        for t in range(NT):
            nc.sync.dma_start(X[:, t], img_t[:, t])

        # ---- helpers -----------------------------------------------------
        def load_whd(nm, tag):
            """Full [H, D, HS] weight -> fp8 [128 dpart, ND, H*HS], cast
            in-flight by the software-DGE DMA (one DMA per d-tile)."""
            w8 = big.tile([P, ND, D], F8, tag=tag)
            for dt in range(ND):
                nc.gpsimd.dma_start(
                    w8[:, dt, :].rearrange("p (h e) -> p h e", e=HS),
                    dw[nm].ap().rearrange("h (dt p) e -> dt p h e", p=P)[dt])
            return w8

        def layernorm_to_T(i, XHT):
            for t in range(NT):
                st = stats.tile([P, 3, 6], F32, tag="bst")
                xg = X[:, t, :].rearrange("p (g d) -> p g d", d=256)
                for g in range(3):
                    nc.vector.bn_stats(st[:, g, :], xg[:, g, :])
                mv = stats.tile([P, 2], F32, tag="mv")
                nc.vector.bn_aggr(mv[:], st[:])
                sd = stats.tile([P, 1], F32, tag="sd")
                nc.scalar.activation(sd[:], mv[:, 1:2], AF.Sqrt, bias=eps_t[:])
                rstd = stats.tile([P, 1], F32, tag="rstd")
                nc.vector.reciprocal(rstd[:], sd[:])
                nmr = stats.tile([P, 1], F32, tag="nmr")
                nc.vector.tensor_scalar(nmr[:], mv[:, 0:1], rstd[:], -1.0,
                                        OP.mult, OP.mult)
                if i in ln_w_t or i in ln_b_t:
                    xf = small.tile([P, D], F32, tag="fst")
                    nc.vector.tensor_scalar(xf[:], X[:, t, :], mv[:, 0:1],
                                            rstd[:], OP.subtract, OP.mult)
                    xh = small.tile([P, D], BF16, tag="xh")
                    if i in ln_w_t and i in ln_b_t:
                        nc.vector.tensor_mul(xf[:], xf[:], ln_w_t[i][:])
                        nc.vector.tensor_tensor(xh[:], xf[:], ln_b_t[i][:], OP.add)
                    elif i in ln_w_t:
                        nc.vector.tensor_tensor(xh[:], xf[:], ln_w_t[i][:], OP.mult)
                    else:
                        nc.vector.tensor_tensor(xh[:], xf[:], ln_b_t[i][:], OP.add)
                else:
                    xh = small.tile([P, D], BF16, tag="xh")
                    nc.scalar.activation(xh[:], X[:, t, :], AF.Identity,
                                         bias=nmr[:], scale=rstd[:])
                pt = ps.tile([P, D], BF16, tag=("sA" if t % 2 == 0 else "sB"))
                for j in range(ND):
                    nc.tensor.transpose(pt[:, j * P:(j + 1) * P],
                                        xh[:, j * P:(j + 1) * P], idb[:])
                nc.vector.tensor_copy(
                    XHT[:, :, t * P:(t + 1) * P],
                    pt[:].rearrange("p (j q) -> p j q", q=P))

        def project_v(wv, XT, n_tok, dest, bias_t):
            """dest [P tok, n_tok//P, H, HS+4] fp8: per-head V plus a ones
            column (65th) so the AV matmul also produces the softmax
            denominator at output partition 64."""
            nc.vector.memset(dest[:, :, :, HS:HS + 1], 1.0)
            nc.vector.memset(dest[:, :, :, HS + 1:HS + 4], 0.0)
            for t in range(n_tok // P):
                for o, w, tg in ((0, 512, "po0"), (512, 256, "po1")):
                    pv = ps.tile([P, w], F32, tag=tg)
                    for dt2 in range(ND // 2):
                        nc.tensor.matmul(
                            pv[:],
                            XT[:, 2 * dt2:2 * dt2 + 2, t * P:(t + 1) * P],
                            wv[:, 2 * dt2:2 * dt2 + 2, o:o + w],
                            start=(dt2 == 0), stop=(dt2 == ND // 2 - 1),
                            perf_mode=DR)
                    nh = w // HS
                    dv = dest[:, t, o // HS:o // HS + nh, 0:HS]
                    pvh = pv[:].rearrange("p (h e) -> p h e", e=HS)
                    if bias_t is not None:
                        bt = bias_t[:, o:o + w].rearrange(
                            "p (h e) -> p h e", e=HS)
                        nc.vector.tensor_tensor(dv, pvh, bt, OP.add)
                    else:
                        nc.vector.tensor_copy(dv, pvh)

        def attention(wq8, wk8, XT, KXT, n_kv, Vt, qb, kb, use_act=False):
            """Full attention pass; adds output into X in place."""
            nk = n_kv // P
            pending = []

            def flush_attn_out(g, ao8):
                # ao8 [65, 2, T] bf16: rows 0-63 unnormalized head output,
                # row 64 softmax denominator. Transpose token-major, then
                # normalize with per-token (per-partition) reciprocals.
                for i in range(2):
                    for c in range(2):
                        att = ps.tile([P, 4, HS + 2], BF16,
                                      tag=("po0" if (2 * i + c) % 2 == 0
                                           else "po1"))
                        for t4 in range(4):
                            t = c * 4 + t4
                            nc.tensor.transpose(
                                att[:, t4, 0:HS + 1],
                                ao8[0:HS + 1, i, t * P:(t + 1) * P],
                                idb[0:HS + 1, 0:HS + 1])
                        rec = stats.tile([P, 4], F32, tag="rec4")
                        nc.vector.reciprocal(rec[:], att[:, :, HS])
                        tmp = small.tile([P, 4, HS], BF16, tag="tmp")
                        for t4 in range(4):
                            nc.vector.tensor_scalar(
                                tmp[:, t4, :], att[:, t4, 0:HS],
                                rec[:, t4:t4 + 1], None, OP.mult)
                        col = g * P + i * HS
                        xv = X[:, c * 4:(c + 1) * 4, col:col + HS]
                        nc.vector.tensor_tensor(xv, tmp[:], xv, OP.add)

            def do_proj(g):
                pq = ps.tile([P, T], F32, tag="pq")
                for c in range(2):
                    for dt2 in range(ND // 2):
                        nc.tensor.matmul(
                            pq[:, c * 512:(c + 1) * 512],
                            wq8[:, 2 * dt2:2 * dt2 + 2, g * P:(g + 1) * P],
                            XT[:, 2 * dt2:2 * dt2 + 2, c * 512:(c + 1) * 512],
                            start=(dt2 == 0), stop=(dt2 == ND // 2 - 1),
                            perf_mode=DR)
                qg = small.tile([P, T], BF16, tag="qg")
                if qb is not None:
                    nc.vector.tensor_scalar(qg[:], pq[:],
                                            qb[:, g:g + 1], None, OP.add)
                elif use_act:
                    nc.scalar.copy(qg[:], pq[:])
                else:
                    nc.vector.tensor_copy(qg[:], pq[:])
                pk = ps.tile([P, n_kv], F32, tag="pq")
                for c in range(max(1, n_kv // 512)):
                    w = min(512, n_kv)
                    for dt2 in range(ND // 2):
                        nc.tensor.matmul(
                            pk[:, c * w:(c + 1) * w],
                            wk8[:, 2 * dt2:2 * dt2 + 2, g * P:(g + 1) * P],
                            KXT[:, 2 * dt2:2 * dt2 + 2, c * w:(c + 1) * w],
                            start=(dt2 == 0), stop=(dt2 == ND // 2 - 1),
                            perf_mode=DR)
                kg = small.tile([P, n_kv], BF16, tag="kg")
                if kb is not None:
                    nc.vector.tensor_scalar(kg[:], pk[:],
                                            kb[:, g:g + 1], None, OP.add)
                elif use_act:
                    nc.scalar.copy(kg[:], pk[:])
                else:
                    nc.vector.tensor_copy(kg[:], pk[:])
                return qg, kg

            carry = do_proj(0)
            for g in range(NP):
                qg, kg = carry

                if nk >= 4:
                    bounds = [2, 2, nk - 4]
                else:
                    bounds = [nk, 0, 0]
                tags = ["exphC", "exphB", "exph"]
                offs = [0, bounds[0], bounds[0] + bounds[1]]
                exs = []
                for j in range(3):
                    if bounds[j] > 0:
                        exs.append(big.tile([P, 2, bounds[j], T], F8,
                                            tag=tags[j], name=f"ex{j}_{g}"))
                    else:
                        exs.append(None)

                def exidx(k):
                    j = 0 if k < offs[1] else (1 if k < offs[2] else 2)
                    return j, k - offs[j]
                for k in range(nk):
                    for i in range(2):
                        eh, ek = exidx(k)
                        sc = ps.tile(
                            [P, T], F32,
                            tag=("sA" if (2 * k + i) % 2 == 0 else "sB"))
                        for c in range(2):
                            nc.tensor.matmul(
                                sc[:, c * 512:(c + 1) * 512],
                                kg[i * HS:(i + 1) * HS, k * P:(k + 1) * P],
                                qg[i * HS:(i + 1) * HS, c * 512:(c + 1) * 512],
                                start=True, stop=True)
                        nc.scalar.activation(exs[eh][:, i, ek, :], sc[:],
                                             AF.Exp, scale=SCALE)

                if g + 1 < NP:
                    carry = do_proj(g + 1)
                while len(pending) > 0:
                    flush_attn_out(*pending.pop(0))

                ao8 = small.tile([P, 2, T], BF16, tag="ao8")
                for c in range(2):
                    for i in range(2):
                        po = ps.tile([P, 512], F32,
                                     tag=("po0" if (2 * c + i) % 2 == 0
                                          else "po1"))
                        for kp in range(nk // 2):
                            eh, ek = exidx(2 * kp)
                            nc.tensor.matmul(
                                po[0:HS + 4, :],
                                Vt[:, 2 * kp:2 * kp + 2, 2 * g + i, 0:HS + 4],
                                exs[eh][:, i, ek:ek + 2,
                                        c * 512:(c + 1) * 512],
                                start=(kp == 0), stop=(kp == nk // 2 - 1),
                                perf_mode=DR)
                        nc.vector.tensor_copy(
                            ao8[0:HS + 1, i, c * 512:(c + 1) * 512],
                            po[0:HS + 1, :])

                pending.append((g, ao8))

            while pending:
                flush_attn_out(*pending.pop(0))

        # =================== weight preloads (issue order = need order) ==
        wq_s = load_whd("sWq", "wqs")
        wk_s = load_whd("sWk", "wks")
        wv_s = load_whd("sWv", "wvs")
        wq_c = load_whd("cWq", "wqc")
        wk_c = load_whd("cWk", "wkc")
        wv_c = load_whd("cWv", "wvc")
        w1f = big.tile([P, ND, FF], F8, tag="w1")
        nc.gpsimd.dma_start(
            w1f[:], dw["W1"].ap().rearrange("(dt p) c -> p dt c", p=P))
        W2b = big.tile([P, NF, D], BF16, tag="w2")
        nc.gpsimd.dma_start(
            W2b[:], dw["W2"].ap().rearrange("(f p) d -> p f d", p=P))

        # =================== self attention ==============================
        XHT = big.tile([P, ND, T], F8, tag="xht")
        layernorm_to_T(1, XHT)
        V = big.tile([P, NT, H, HS + 4], F8, tag="vw2")
        project_v(wv_s, XHT, T, V, sbv_t)
        attention(wq_s, wk_s, XHT, XHT, T, V, sbq_t, sbk_t)

        # =================== cross attention =============================
        XHT2 = big.tile([P, ND, T], F8, tag="xht")
        layernorm_to_T(2, XHT2)
        Vc = big.tile([P, NTC, H, HS + 4], F8, tag="vw2")
        project_v(wv_c, ctxT, TC, Vc, cbv_t)
        attention(wq_c, wk_c, XHT2, ctxT, TC, Vc, cbq_t, cbk_t, use_act=True)

        # =================== FFN =========================================
        XHT3 = big.tile([P, ND, T], F8, tag="xht")
        layernorm_to_T(3, XHT3)

        H2 = big.tile([P, NF, T], BF16, tag="exph")
        for f in range(NF):
            ph = ps.tile([P, T], F32, tag=("sA" if f % 2 == 0 else "pq"))
            for c in range(2):
                for dt2 in range(ND // 2):
                    nc.tensor.matmul(
                        ph[:, c * 512:(c + 1) * 512],
                        w1f[:, 2 * dt2:2 * dt2 + 2, f * P:(f + 1) * P],
                        XHT3[:, 2 * dt2:2 * dt2 + 2, c * 512:(c + 1) * 512],
                        start=(dt2 == 0), stop=(dt2 == ND // 2 - 1),
                        perf_mode=DR)
            nc.scalar.activation(
                H2[:, f, :], ph[:], AF.Silu,
                bias=(b1_t[:, f:f + 1] if b1_t is not None else 0.0))


        for t in range(NT):
            pf = ps.tile([P, D], F32, tag=("sB" if t % 2 == 0 else "pq"))
            for o, w in ((0, 512), (512, 256)):
                for f in range(NF):
                    nc.tensor.matmul(
                        pf[:, o:o + w],
                        H2[:, f, t * P:(t + 1) * P],
                        W2b[:, f, o:o + w],
                        start=(f == 0), stop=(f == NF - 1))
            ot = small.tile([P, D], F32, tag="ot")
            nc.vector.tensor_tensor(ot[:], pf[:], X[:, t, :], OP.add)
            if b2_t is not None:
                nc.vector.tensor_add(ot[:], ot[:], b2_t[:])
            nc.sync.dma_start(out_ap[:, t], ot[:])

    nc.compile()
    return nc


_CACHE = {}


def _flags_of(inputs):
    f = {}
    for i in (1, 2, 3):
        f[f"ln{i}_w_triv"] = bool(np.all(inputs[f"ln{i}_w"] == 1.0))
        f[f"ln{i}_b_triv"] = bool(np.all(inputs[f"ln{i}_b"] == 0.0))
    for nm in ["sbq", "sbk", "sbv", "cbq", "cbk", "cbv", "b1", "b2"]:
        f[f"{nm}_zero"] = bool(np.all(inputs[nm] == 0.0))
    return f


def kernel(**inputs):
    flags = _flags_of(inputs)
    key = tuple(sorted(flags.items()))
    if key not in _CACHE:
        _CACHE[key] = _build(flags)
    nc = _CACHE[key]

    in_maps = []
    for b in range(B):
        m = {"img_embedding": np.ascontiguousarray(
                 inputs["img_embedding"][b].astype(np.float32)),
             "context": np.ascontiguousarray(
                 inputs["context"][b].astype(np.float32))}
        for nm in WEIGHT_NAMES:
            m[nm] = np.ascontiguousarray(inputs[nm].astype(np.float32))
        in_maps.append(m)

    res = run_bass_kernel_spmd(nc, in_maps, core_ids=list(range(B)))
    return np.stack([res.results[b]["out"] for b in range(B)], axis=0)



# revision 41
# speedup vs baseline: 1.3461x; 1.0389x over previous
"""DiT block kernel for Trainium2 (Bass/Tile), data-parallel over batch on 8 cores.

Per-core dataflow (one batch element per core; no collectives needed):
  - residual stream X [128 tok, 8, 768] fp32 in SBUF, updated in place
  - LayerNorm token-major (bn_stats/bn_aggr) -> xhat bf16 -> PE-transpose
    (batched per token tile) to feature-major XHT [128 d, 6, 1024 tok] bf16
  - per head-pair (2 heads x 64 hs = 128 partitions): Q then K projected with
    bf16 matmuls on a dedicated psum tag, evicted bf16
  - scoresT per (k-tile, head): bf16 row-located matmuls (head A partitions
    0-63, head B 64-127, auto row tile position) into ping-ponged psum tiles
    -> exp on ACT (scale=1/8 folded in; no max subtraction - logits are O(1)
    by construction) -> bf16
  - exp@V and the softmax denominator (all-ones lhsT) col-packed per head
    pair into one [128, 1024] psum (tile_position=(0, 64) for head B);
    normalize via DVE reciprocal+mul; PE-transpose back (batched, deferred
    one pair for overlap); residual added into X in place
  - FFN: h1 feature-major bf16 per ff tile on ping-ponged psum, Silu on ACT
    -> H2 bf16 resident; W2 cast to bf16 ahead of time (gpsimd); second
    matmul bf16; residual fused into the psum eviction
  - weights stream from HBM in chunks (f32) and are cast to bf16 on
    gpsimd/DVE off the critical path; fp32 accumulation everywhere in PSUM
"""

import os
import sys

import numpy as np

for _p in ("/opt/trn_rl_repo", "/root/.axon_site/_ro/trn_rl_repo"):
    if os.path.isdir(_p) and _p not in sys.path:
        sys.path.insert(0, _p)

import concourse.bass as bass
import concourse.mybir as mybir
import concourse.tile as tile
from concourse import bacc
from concourse.bass_utils import run_bass_kernel_spmd
from concourse.masks import make_identity

F32 = mybir.dt.float32
F32R = mybir.dt.float32r
BF16 = mybir.dt.bfloat16
F8 = mybir.dt.float8e4
AF = mybir.ActivationFunctionType
OP = mybir.AluOpType
DR = mybir.MatmulPerfMode.DoubleRow

B, T, TC, D, H, HS, FF = 8, 1024, 768 // 3, 768, 12, 64, 3072
P = 128
NT = T // P      # 8 token tiles
NTC = TC // P    # 2 context token tiles
ND = D // P      # 6 feature tiles
NF = FF // P     # 24 ffn tiles
NP = H // 2      # 6 head pairs
EPS = 1e-5
SCALE = HS ** -0.5

WEIGHT_NAMES = [
    "ln1_w", "ln1_b", "sWq", "sbq", "sWk", "sbk", "sWv", "sbv",
    "ln2_w", "ln2_b", "cWq", "cbq", "cWk", "cbk", "cWv", "cbv",
    "ln3_w", "ln3_b", "W1", "b1", "W2", "b2",
]


def _build(flags):
    nc = bacc.Bacc("TRN2", target_bir_lowering=False, debug=False)

    d_img = nc.dram_tensor("img_embedding", [T, D], F32, kind="ExternalInput")
    d_ctx = nc.dram_tensor("context", [TC, D], F32, kind="ExternalInput")
    dw = {}
    for i in (1, 2, 3):
        dw[f"ln{i}_w"] = nc.dram_tensor(f"ln{i}_w", [D], F32, kind="ExternalInput")
        dw[f"ln{i}_b"] = nc.dram_tensor(f"ln{i}_b", [D], F32, kind="ExternalInput")
    for nm in ["sWq", "sWk", "sWv", "cWq", "cWk", "cWv"]:
        # host pre-permutes [H, D, HS] -> [D, H*HS] for long-run DMAs
        dw[nm] = nc.dram_tensor(nm, [D, H * HS], F32, kind="ExternalInput")
    for nm in ["sbq", "sbk", "sbv", "cbq", "cbk", "cbv"]:
        dw[nm] = nc.dram_tensor(nm, [H, HS], F32, kind="ExternalInput")
    dw["W1"] = nc.dram_tensor("W1", [D, FF], F32, kind="ExternalInput")
    dw["b1"] = nc.dram_tensor("b1", [FF], F32, kind="ExternalInput")
    dw["W2"] = nc.dram_tensor("W2", [FF, D], F32, kind="ExternalInput")
    dw["b2"] = nc.dram_tensor("b2", [D], F32, kind="ExternalInput")
    d_out = nc.dram_tensor("out", [T, D], F32, kind="ExternalOutput")
    out_ap = d_out.ap().rearrange("(n p) d -> p n d", p=P)

    with tile.TileContext(nc) as tc, (
        tc.tile_pool(name="const", bufs=1)
    ) as const, (
        tc.tile_pool(name="resid", bufs=1)
    ) as resid, (
        tc.tile_pool(name="wpool", bufs=2)
    ) as wpool, (
        tc.tile_pool(name="big", bufs=1)
    ) as big, (
        tc.tile_pool(name="small", bufs=2)
    ) as small, (
        tc.tile_pool(name="stats", bufs=3)
    ) as stats, (
        tc.tile_pool(name="ps", bufs=1, space="PSUM")
    ) as ps:

        # ---- constants ---------------------------------------------------
        idb = const.tile([P, P], BF16)
        make_identity(nc, idb)
        eps_t = const.tile([P, 1], F32)
        nc.vector.memset(eps_t[:], EPS)

        def bcast_row(dram_ap, n):
            t = const.tile([P, n], F32)
            src = bass.AP(tensor=dram_ap.tensor, offset=dram_ap.offset,
                          ap=[[0, P]] + list(dram_ap.ap))
            nc.gpsimd.dma_start(t[:], src)
            return t

        ln_w_t, ln_b_t = {}, {}
        for i in (1, 2, 3):
            if not flags[f"ln{i}_w_triv"]:
                ln_w_t[i] = bcast_row(dw[f"ln{i}_w"].ap(), D)
            if not flags[f"ln{i}_b_triv"]:
                ln_b_t[i] = bcast_row(dw[f"ln{i}_b"].ap(), D)
        b2_t = None if flags["b2_zero"] else bcast_row(dw["b2"].ap(), D)

        def pair_bias(nm):
            t = const.tile([P, NP], F32)
            nc.sync.dma_start(
                t[:], dw[nm].ap().rearrange("(g i) e -> (i e) g", i=2))
            return t

        sbq_t = None if flags["sbq_zero"] else pair_bias("sbq")
        sbk_t = None if flags["sbk_zero"] else pair_bias("sbk")
        cbq_t = None if flags["cbq_zero"] else pair_bias("cbq")
        cbk_t = None if flags["cbk_zero"] else pair_bias("cbk")
        sbv_t = None if flags["sbv_zero"] else bcast_row(
            dw["sbv"].ap().rearrange("h e -> (h e)"), D)
        cbv_t = None if flags["cbv_zero"] else bcast_row(
            dw["cbv"].ap().rearrange("h e -> (h e)"), D)
        b1_t = None
        if not flags["b1_zero"]:
            b1_t = const.tile([P, NF], F32)
            nc.sync.dma_start(b1_t[:], dw["b1"].ap().rearrange("(f p) -> p f", p=P))

        # ---- residual stream + context (transposed, fp8) ----------------
        ctxT = resid.tile([P, ND, TC], F8)
        for t in range(NTC):
            cbf = small.tile([P, D], BF16, tag="xh")
            nc.gpsimd.dma_start(cbf[:], d_ctx.ap().rearrange(
                "(n p) d -> p n d", p=P)[:, t])
            pt = ps.tile([P, D], BF16, tag=("sA" if t % 2 == 0 else "sB"))
            for j in range(ND):
                nc.tensor.transpose(pt[:, j * P:(j + 1) * P],
                                    cbf[:, j * P:(j + 1) * P], idb[:])
            nc.vector.tensor_copy(ctxT[:, :, t * P:(t + 1) * P], pt[:].rearrange(
                "p (j q) -> p j q", q=P))

        X = resid.tile([P, NT, D], F32)
        img_t = d_img.ap().rearrange("(n p) d -> p n d", p=P)
        for t in range(NT):
            nc.sync.dma_start(X[:, t], img_t[:, t])

        # ---- helpers -----------------------------------------------------
        def load_whd(nm, tag):
            """Full [H, D, HS] weight -> fp8 [128 dpart, ND, H*HS], cast
            in-flight by the software-DGE DMA (one DMA per d-tile)."""
            w8 = big.tile([P, ND, D], F8, tag=tag)
            nc.gpsimd.dma_start(
                w8[:], dw[nm].ap().rearrange("(dt p) c -> p dt c", p=P))
            return w8

        def layernorm_to_T(i, XHT):
            for t in range(NT):
                st = stats.tile([P, 3, 6], F32, tag="bst")
                xg = X[:, t, :].rearrange("p (g d) -> p g d", d=256)
                for g in range(3):
                    nc.vector.bn_stats(st[:, g, :], xg[:, g, :])
                mv = stats.tile([P, 2], F32, tag="mv")
                nc.vector.bn_aggr(mv[:], st[:])
                sd = stats.tile([P, 1], F32, tag="sd")
                nc.scalar.activation(sd[:], mv[:, 1:2], AF.Sqrt, bias=eps_t[:])
                rstd = stats.tile([P, 1], F32, tag="rstd")
                nc.vector.reciprocal(rstd[:], sd[:])
                nmr = stats.tile([P, 1], F32, tag="nmr")
                nc.vector.tensor_scalar(nmr[:], mv[:, 0:1], rstd[:], -1.0,
                                        OP.mult, OP.mult)
                if i in ln_w_t or i in ln_b_t:
                    xf = small.tile([P, D], F32, tag="fst")
                    nc.vector.tensor_scalar(xf[:], X[:, t, :], mv[:, 0:1],
                                            rstd[:], OP.subtract, OP.mult)
                    xh = small.tile([P, D], BF16, tag="xh")
                    if i in ln_w_t and i in ln_b_t:
                        nc.vector.tensor_mul(xf[:], xf[:], ln_w_t[i][:])
                        nc.vector.tensor_tensor(xh[:], xf[:], ln_b_t[i][:], OP.add)
                    elif i in ln_w_t:
                        nc.vector.tensor_tensor(xh[:], xf[:], ln_w_t[i][:], OP.mult)
                    else:
                        nc.vector.tensor_tensor(xh[:], xf[:], ln_b_t[i][:], OP.add)
                else:
                    xh = small.tile([P, D], BF16, tag="xh")
                    nc.scalar.activation(xh[:], X[:, t, :], AF.Identity,
                                         bias=nmr[:], scale=rstd[:])
                pt = ps.tile([P, D], BF16, tag=("sA" if t % 2 == 0 else "sB"))
                for j in range(ND):
                    nc.tensor.transpose(pt[:, j * P:(j + 1) * P],
                                        xh[:, j * P:(j + 1) * P], idb[:])
                nc.scalar.copy(
                    XHT[:, :, t * P:(t + 1) * P],
                    pt[:].rearrange("p (j q) -> p j q", q=P))

        def project_v(wv, XT, n_tok, dest, bias_t):
            """dest [P tok, n_tok//P, H, HS+4] fp8: per-head V plus a ones
            column (65th) so the AV matmul also produces the softmax
            denominator at output partition 64."""
            nc.vector.memset(dest[:, :, :, HS:HS + 1], 1.0)
            nc.vector.memset(dest[:, :, :, HS + 1:HS + 4], 0.0)
            for t in range(n_tok // P):
                for o, w, tg in ((0, 512, "po0"), (512, 256, "po1")):
                    pv = ps.tile([P, w], F32, tag=tg)
                    for dt2 in range(ND // 2):
                        nc.tensor.matmul(
                            pv[:],
                            XT[:, 2 * dt2:2 * dt2 + 2, t * P:(t + 1) * P],
                            wv[:, 2 * dt2:2 * dt2 + 2, o:o + w],
                            start=(dt2 == 0), stop=(dt2 == ND // 2 - 1),
                            perf_mode=DR)
                    nh = w // HS
                    dv = dest[:, t, o // HS:o // HS + nh, 0:HS]
                    pvh = pv[:].rearrange("p (h e) -> p h e", e=HS)
                    if bias_t is not None:
                        bt = bias_t[:, o:o + w].rearrange(
                            "p (h e) -> p h e", e=HS)
                        nc.vector.tensor_tensor(dv, pvh, bt, OP.add)
                    else:
                        nc.scalar.copy(dv, pvh)

        def attention(wq8, wk8, XT, KXT, n_kv, Vt, qb, kb, use_act=False):
            """Full attention pass; adds output into X in place."""
            nk = n_kv // P
            pending = []

            def flush_attn_out(g, ao8):
                # ao8 [65, 2, T] bf16: rows 0-63 unnormalized head output,
                # row 64 softmax denominator. Transpose token-major, then
                # normalize with per-token (per-partition) reciprocals.
                for i in range(2):
                    for c in range(2):
                        att = ps.tile([P, 4, HS + 2], BF16,
                                      tag=("po0" if (2 * i + c) % 2 == 0
                                           else "po1"))
                        for t4 in range(4):
                            t = c * 4 + t4
                            nc.tensor.transpose(
                                att[:, t4, 0:HS + 1],
                                ao8[0:HS + 1, i, t * P:(t + 1) * P],
                                idb[0:HS + 1, 0:HS + 1])
                        rec = stats.tile([P, 4], F32, tag="rec4")
                        nc.vector.reciprocal(rec[:], att[:, :, HS])
                        tmp = small.tile([P, 4, HS], BF16, tag="tmp")
                        for t4 in range(4):
                            nc.vector.tensor_scalar(
                                tmp[:, t4, :], att[:, t4, 0:HS],
                                rec[:, t4:t4 + 1], None, OP.mult)
                        col = g * P + i * HS
                        xv = X[:, c * 4:(c + 1) * 4, col:col + HS]
                        nc.gpsimd.tensor_tensor(xv, tmp[:], xv, OP.add)

            def do_proj(g):
                pq = ps.tile([P, T], F32, tag="pq")
                for c in range(2):
                    for dt2 in range(ND // 2):
                        nc.tensor.matmul(
                            pq[:, c * 512:(c + 1) * 512],
                            wq8[:, 2 * dt2:2 * dt2 + 2, g * P:(g + 1) * P],
                            XT[:, 2 * dt2:2 * dt2 + 2, c * 512:(c + 1) * 512],
                            start=(dt2 == 0), stop=(dt2 == ND // 2 - 1),
                            perf_mode=DR)
                qg = small.tile([P, T], BF16, tag="qg")
                if qb is not None:
                    nc.vector.tensor_scalar(qg[:], pq[:],
                                            qb[:, g:g + 1], None, OP.add)
                elif use_act:
                    nc.scalar.copy(qg[:], pq[:])
                else:
                    nc.vector.tensor_copy(qg[:], pq[:])
                pk = ps.tile([P, n_kv], F32, tag="pq")
                for c in range(max(1, n_kv // 512)):
                    w = min(512, n_kv)
                    for dt2 in range(ND // 2):
                        nc.tensor.matmul(
                            pk[:, c * w:(c + 1) * w],
                            wk8[:, 2 * dt2:2 * dt2 + 2, g * P:(g + 1) * P],
                            KXT[:, 2 * dt2:2 * dt2 + 2, c * w:(c + 1) * w],
                            start=(dt2 == 0), stop=(dt2 == ND // 2 - 1),
                            perf_mode=DR)
                kg = small.tile([P, n_kv], BF16, tag="kg")
                if kb is not None:
                    nc.vector.tensor_scalar(kg[:], pk[:],
                                            kb[:, g:g + 1], None, OP.add)
                elif use_act:
                    nc.scalar.copy(kg[:], pk[:])
                else:
                    nc.vector.tensor_copy(kg[:], pk[:])
                return qg, kg

            carry = do_proj(0)
            for g in range(NP):
                qg, kg = carry

                if nk >= 4:
                    bounds = [2, 2, nk - 4]
                else:
                    bounds = [nk, 0, 0]
                tags = ["exphC", "exphB", "exph"]
                offs = [0, bounds[0], bounds[0] + bounds[1]]
                exs = []
                for j in range(3):
                    if bounds[j] > 0:
                        exs.append(big.tile([P, 2, bounds[j], T], F8,
                                            tag=tags[j], name=f"ex{j}_{g}"))
                    else:
                        exs.append(None)

                def exidx(k):
                    j = 0 if k < offs[1] else (1 if k < offs[2] else 2)
                    return j, k - offs[j]
                for k in range(nk):
                    for i in range(2):
                        eh, ek = exidx(k)
                        sc = ps.tile(
                            [P, T], F32,
                            tag=("sA" if (2 * k + i) % 2 == 0 else "sB"))
                        for c in range(2):
                            nc.tensor.matmul(
                                sc[:, c * 512:(c + 1) * 512],
                                kg[i * HS:(i + 1) * HS, k * P:(k + 1) * P],
                                qg[i * HS:(i + 1) * HS, c * 512:(c + 1) * 512],
                                start=True, stop=True)
                        nc.scalar.activation(exs[eh][:, i, ek, :], sc[:],
                                             AF.Exp, scale=SCALE)

                if g + 1 < NP:
                    carry = do_proj(g + 1)
                while len(pending) > 0:
                    flush_attn_out(*pending.pop(0))

                ao8 = small.tile([P, 2, T], BF16, tag="ao8")
                for c in range(2):
                    for i in range(2):
                        po = ps.tile([P, 512], F32,
                                     tag=("po0" if (2 * c + i) % 2 == 0
                                          else "po1"))
                        for kp in range(nk // 2):
                            eh, ek = exidx(2 * kp)
                            nc.tensor.matmul(
                                po[0:HS + 4, :],
                                Vt[:, 2 * kp:2 * kp + 2, 2 * g + i, 0:HS + 4],
                                exs[eh][:, i, ek:ek + 2,
                                        c * 512:(c + 1) * 512],
                                start=(kp == 0), stop=(kp == nk // 2 - 1),
                                perf_mode=DR)
                        if use_act and c == 1:
                            nc.scalar.copy(
                                ao8[0:HS + 1, i, c * 512:(c + 1) * 512],
                                po[0:HS + 1, :])
                        else:
                            nc.vector.tensor_copy(
                                ao8[0:HS + 1, i, c * 512:(c + 1) * 512],
                                po[0:HS + 1, :])

                pending.append((g, ao8))

            while pending:
                flush_attn_out(*pending.pop(0))

        # =================== weight preloads (issue order = need order) ==
        wq_s = load_whd("sWq", "wqs")
        wk_s = load_whd("sWk", "wks")
        wv_s = load_whd("sWv", "wvs")
        wq_c = load_whd("cWq", "wqc")
        wk_c = load_whd("cWk", "wkc")
        wv_c = load_whd("cWv", "wvc")
        w1f = big.tile([P, ND, FF], F8, tag="w1")
        nc.gpsimd.dma_start(
            w1f[:], dw["W1"].ap().rearrange("(dt p) c -> p dt c", p=P))
        W2b = big.tile([P, NF, D], BF16, tag="w2")
        nc.gpsimd.dma_start(
            W2b[:], dw["W2"].ap().rearrange("(f p) d -> p f d", p=P))

        # =================== self attention ==============================
        XHT = big.tile([P, ND, T], F8, tag="xht")
        layernorm_to_T(1, XHT)
        V = big.tile([P, NT, H, HS + 4], F8, tag="vw2")
        project_v(wv_s, XHT, T, V, sbv_t)
        attention(wq_s, wk_s, XHT, XHT, T, V, sbq_t, sbk_t)

        # =================== cross attention =============================
        XHT2 = big.tile([P, ND, T], F8, tag="xht")
        layernorm_to_T(2, XHT2)
        Vc = big.tile([P, NTC, H, HS + 4], F8, tag="vw2")
        project_v(wv_c, ctxT, TC, Vc, cbv_t)
        attention(wq_c, wk_c, XHT2, ctxT, TC, Vc, cbq_t, cbk_t, use_act=True)

        # =================== FFN =========================================
        XHT3 = big.tile([P, ND, T], F8, tag="xht")
        layernorm_to_T(3, XHT3)

        H2 = big.tile([P, NF, T], BF16, tag="exph")
        pf0 = ps.tile([P, D], F32, tag="sB")
        for f in range(NF):
            ph = ps.tile([P, T], F32, tag=("sA" if f % 2 == 0 else "pq"))
            for c in range(2):
                for dt2 in range(ND // 2):
                    nc.tensor.matmul(
                        ph[:, c * 512:(c + 1) * 512],
                        w1f[:, 2 * dt2:2 * dt2 + 2, f * P:(f + 1) * P],
                        XHT3[:, 2 * dt2:2 * dt2 + 2, c * 512:(c + 1) * 512],
                        start=(dt2 == 0), stop=(dt2 == ND // 2 - 1),
                        perf_mode=DR)
            nc.scalar.activation(
                H2[:, f, :], ph[:], AF.Silu,
                bias=(b1_t[:, f:f + 1] if b1_t is not None else 0.0))
            for o, w in ((0, 512), (512, 256)):
                nc.tensor.matmul(
                    pf0[:, o:o + w], H2[:, f, 0:P], W2b[:, f, o:o + w],
                    start=(f == 0), stop=(f == NF - 1))


        ot = small.tile([P, D], F32, tag="ot")
        nc.vector.tensor_tensor(ot[:], pf0[:], X[:, 0, :], OP.add)
        if b2_t is not None:
            nc.vector.tensor_add(ot[:], ot[:], b2_t[:])
        nc.sync.dma_start(out_ap[:, 0], ot[:])
        for t in range(1, NT):
            pf = ps.tile([P, D], F32, tag=("sA" if t % 2 == 0 else "pq"))
            for o, w in ((0, 512), (512, 256)):
                for f in range(NF):
                    nc.tensor.matmul(
                        pf[:, o:o + w],
                        H2[:, f, t * P:(t + 1) * P],
                        W2b[:, f, o:o + w],
                        start=(f == 0), stop=(f == NF - 1))
            ot = small.tile([P, D], F32, tag="ot")
            nc.vector.tensor_tensor(ot[:], pf[:], X[:, t, :], OP.add)
            if b2_t is not None:
                nc.vector.tensor_add(ot[:], ot[:], b2_t[:])
            nc.sync.dma_start(out_ap[:, t], ot[:])

    nc.compile()
    return nc


_CACHE = {}


def _flags_of(inputs):
    f = {}
    for i in (1, 2, 3):
        f[f"ln{i}_w_triv"] = bool(np.all(inputs[f"ln{i}_w"] == 1.0))
        f[f"ln{i}_b_triv"] = bool(np.all(inputs[f"ln{i}_b"] == 0.0))
    for nm in ["sbq", "sbk", "sbv", "cbq", "cbk", "cbv", "b1", "b2"]:
        f[f"{nm}_zero"] = bool(np.all(inputs[nm] == 0.0))
    return f


def kernel(**inputs):
    flags = _flags_of(inputs)
    key = tuple(sorted(flags.items()))
    if key not in _CACHE:
        _CACHE[key] = _build(flags)
    nc = _CACHE[key]

    in_maps = []
    for b in range(B):
        m = {"img_embedding": np.ascontiguousarray(
                 inputs["img_embedding"][b].astype(np.float32)),
             "context": np.ascontiguousarray(
                 inputs["context"][b].astype(np.float32))}
        for nm in WEIGHT_NAMES:
            a = inputs[nm].astype(np.float32)
            if nm in ("sWq", "sWk", "sWv", "cWq", "cWk", "cWv"):
                a = a.transpose(1, 0, 2).reshape(D, H * HS)
            m[nm] = np.ascontiguousarray(a)
        in_maps.append(m)

    res = run_bass_kernel_spmd(nc, in_maps, core_ids=list(range(B)))
    return np.stack([res.results[b]["out"] for b in range(B)], axis=0)

